# revision 8
# baseline (speedup 1.0000x reference)
"""Bass/Trainium2 kernel for BasicBiPointNetSemSeg (binarized PointNet semantic seg).

Data-parallel over 8 NeuronCores: batch 16 -> 2 point clouds per core.
Layout on device: channels on partitions, points on the free dim.

Key transformations (all exact, not approximations):
  - sign(W) precomputed on host, stored transposed as bf16 (+-1 exact in bf16).
  - sign(x) @ sign(W).T done as bf16 matmuls (integer accumulation, exact).
  - BatchNorm + bilinear bias folded into one affine (scale s>0, bias c) applied
    by the ScalarE activation: Sign(z*s + c) produces the next layer's +-1 input
    in one op.  ht (clip) before a sign is dropped: sign(clip(x)) == sign(x).
  - Max-pool layers (stn c3 / fstn c3 / enc c3): pool the RAW integer matmul
    outputs (monotone affine with s>0 commutes with max), apply the affine once
    per cloud after pooling.  Raw values are integers |z|<=128 -> bf16 exact.
  - Global feature g enters the seg head only via sign(g) @ Wg: that term is
    constant per cloud, computed once and folded into the head-c1 bias.
"""

import os
import sys
from contextlib import ExitStack

import numpy as np
import ml_dtypes

for _p in ("/opt/trn_rl_repo",):
    if os.path.isdir(_p) and _p not in sys.path:
        sys.path.append(_p)

import concourse.bacc as bacc
import concourse.bass as bass
import concourse.tile as tile
from concourse import mybir
from concourse.bass_utils import run_bass_kernel_spmd

BF16 = ml_dtypes.bfloat16
F32 = np.float32
DT_BF = mybir.dt.bfloat16
DT_F32 = mybir.dt.float32
AF = mybir.ActivationFunctionType
ALU = mybir.AluOpType
AX = mybir.AxisListType

B, N, NUM_CLASS = 16, 4096, 13
M_CORES = 8
BL = B // M_CORES          # clouds per core
NPTS = BL * N              # points per core
TPTS = 512                 # points per tile
NT = NPTS // TPTS          # tiles per core
TPC = N // TPTS            # tiles per cloud
EPS = 1e-5

# Of the 4 chunk-pairs per 1024-wide pooled layer, how many are reduced by the
# VectorE straight from PSUM (the rest go ScalarE-copy-to-bf16 + VectorE reduce).
DIRECT_PAIRS = 2


# ---------------------------------------------------------------- host prep

def _sgnT(Wdict, kc=None):
    Wt = np.ascontiguousarray(np.sign(np.asarray(Wdict["W"], F32)).T.astype(BF16))
    if kc is not None:
        Wt = np.ascontiguousarray(Wt.reshape(kc, 128, -1))
    return Wt


def _fold(lin, bn):
    g = np.asarray(bn["g"], F32)
    v = np.asarray(bn["v"], F32)
    m = np.asarray(bn["m"], F32)
    be = np.asarray(bn["be"], F32)
    b = np.asarray(lin["b"], F32)
    s = g / np.sqrt(v + EPS)
    c = (b - m) * s + be
    return s.astype(F32), c.astype(F32)


def _chunked(vec, mc):
    """[M] -> [M,1] (mc==1) or [128, mc] with [p, j] = vec[j*128+p]."""
    vec = np.asarray(vec, F32)
    if mc == 1:
        return np.ascontiguousarray(vec.reshape(-1, 1))
    return np.ascontiguousarray(vec.reshape(mc, 128).T)


def _make_wmap(params):
    p = params
    feat = p["feat"]
    stn, fstn = feat["stn"], feat["fstn"]
    w = {}

    def affine(prefix, lin, bn, mc):
        s, c = _fold(lin, bn)
        w[prefix + "_s"] = _chunked(s, mc)
        w[prefix + "_c"] = _chunked(c, mc)

    # --- stn (k=3) ---
    w["stn_c1_w"] = _sgnT(stn["c1"])                 # [9, 64]
    affine("stn1", stn["c1"], stn["b1"], 1)
    w["stn_c2_w"] = _sgnT(stn["c2"])                 # [64, 128]
    affine("stn2", stn["c2"], stn["b2"], 1)
    w["stn_c3_w"] = _sgnT(stn["c3"])                 # [128, 1024]
    affine("stn3", stn["c3"], stn["b3"], 8)
    w["stn_f1_w"] = _sgnT(stn["f1"], kc=8)           # [8,128,512]
    affine("stnf1", stn["f1"], stn["b4"], 4)
    w["stn_f2_w"] = _sgnT(stn["f2"], kc=4)           # [4,128,256]
    affine("stnf2", stn["f2"], stn["b5"], 2)
    w["stn_f3_w"] = _sgnT(stn["f3"], kc=2)           # [2,128,9]
    w["stn_f3_c"] = _chunked(
        np.asarray(stn["f3"]["b"], F32) + np.eye(3, dtype=F32).reshape(-1), 1)

    # --- fstn (k=64) ---
    w["fstn_c1_w"] = _sgnT(fstn["c1"])               # [64, 64]
    affine("fstn1", fstn["c1"], fstn["b1"], 1)
    w["fstn_c2_w"] = _sgnT(fstn["c2"])               # [64, 128]
    affine("fstn2", fstn["c2"], fstn["b2"], 1)
    w["fstn_c3_w"] = _sgnT(fstn["c3"])               # [128, 1024]
    affine("fstn3", fstn["c3"], fstn["b3"], 8)
    w["fstn_f1_w"] = _sgnT(fstn["f1"], kc=8)
    affine("fstnf1", fstn["f1"], fstn["b4"], 4)
    w["fstn_f2_w"] = _sgnT(fstn["f2"], kc=4)
    affine("fstnf2", fstn["f2"], fstn["b5"], 2)
    w["fstn_f3_w"] = _sgnT(fstn["f3"], kc=2)         # [2,128,4096]
    w["fstn_f3_c"] = _chunked(
        np.asarray(fstn["f3"]["b"], F32) + np.eye(64, dtype=F32).reshape(-1), 32)

    # --- encoder ---
    enc1T = _sgnT(feat["c1"])                        # [9, 64]
    w["enc_c1a_w"] = np.ascontiguousarray(enc1T[:3])
    w["enc_c1b_w"] = np.ascontiguousarray(enc1T[3:])
    affine("enc1", feat["c1"], feat["b1"], 1)
    w["enc_c2_w"] = _sgnT(feat["c2"])                # [64, 128]
    affine("enc2", feat["c2"], feat["b2"], 1)
    w["enc_c3_w"] = _sgnT(feat["c3"])                # [128, 1024]
    affine("enc3", feat["c3"], feat["b3"], 8)

    # --- seg head ---
    c1W = np.sign(np.asarray(p["c1"]["W"], F32))     # [512, 1088]
    w["head_c1g_w"] = np.ascontiguousarray(
        c1W[:, :1024].T.astype(BF16).reshape(8, 128, 512))
    w["head_c1p_w"] = np.ascontiguousarray(c1W[:, 1024:].T.astype(BF16))  # [64,512]
    affine("head1", p["c1"], p["b1"], 4)
    w["head_c2_w"] = _sgnT(p["c2"], kc=4)            # [4,128,256]
    affine("head2", p["c2"], p["b2"], 2)
    w["head_c3_w"] = _sgnT(p["c3"], kc=2)            # [2,128,128]
    affine("head3", p["c3"], p["b3"], 1)
    w["c4_wT"] = np.ascontiguousarray(np.asarray(p["c4"]["W"], F32).T)    # [128,13]
    w["c4_b"] = np.ascontiguousarray(np.asarray(p["c4"]["b"], F32).reshape(1, 13))
    return w


# ---------------------------------------------------------------- device program

def _build_program(wmap):
    nc = bacc.Bacc("TRN2", target_bir_lowering=False, debug=False)
    dts = {}
    for name, arr in wmap.items():
        dt = DT_BF if arr.dtype == BF16 else DT_F32
        dts[name] = nc.dram_tensor(name, list(arr.shape), dt, kind="ExternalInput").ap()
    sxT_d = nc.dram_tensor("sxT", [9, NPTS], DT_BF, kind="ExternalInput").ap()
    sfT_d = nc.dram_tensor("sfT", [6, NPTS], DT_BF, kind="ExternalInput").ap()
    xyzT_d = nc.dram_tensor("xyzT", [3, NPTS], DT_F32, kind="ExternalInput").ap()
    lo_d = nc.dram_tensor("logout", [NPTS, NUM_CLASS], DT_F32, kind="ExternalOutput").ap()
    tr_d = nc.dram_tensor("trans_o", [BL, 3, 3], DT_F32, kind="ExternalOutput").ap()
    tf_d = nc.dram_tensor("tf_o", [BL, 64, 64], DT_F32, kind="ExternalOutput").ap()

    with ExitStack() as ctx:
        tc = ctx.enter_context(tile.TileContext(nc))
        wp = ctx.enter_context(tc.tile_pool(name="wpool", bufs=1))
        pp = ctx.enter_context(tc.tile_pool(name="persist", bufs=1))
        sb = ctx.enter_context(tc.tile_pool(name="work", bufs=3))
        ps = ctx.enter_context(tc.tile_pool(name="psum", bufs=3, space="PSUM"))
        psp = ctx.enter_context(tc.tile_pool(name="psumpair", bufs=2, space="PSUM"))
        pss = ctx.enter_context(tc.tile_pool(name="psumsmall", bufs=1, space="PSUM"))
        dr = ctx.enter_context(tc.tile_pool(name="drsc", bufs=1, space="DRAM"))

        sbw = {}

        def w(name):
            if name not in sbw:
                ap = dts[name]
                arr = wmap[name]
                dt = DT_BF if arr.dtype == BF16 else DT_F32
                if arr.ndim == 3:  # [kc, 128, M] -> sbuf [128, kc, M]
                    kc = arr.shape[0]
                    t = wp.tile([128, kc, arr.shape[2]], dt, tag=name)
                    for k in range(kc):
                        nc.sync.dma_start(out=t[:, k, :], in_=ap[k])
                else:
                    t = wp.tile(list(arr.shape), dt, tag=name)
                    nc.sync.dma_start(out=t[:], in_=ap)
                sbw[name] = t
            return sbw[name]

        def mm(dst, lhsT, rhs, start=True, stop=True):
            nc.tensor.matmul(dst, lhsT, rhs, start=start, stop=stop)

        def sgn(dst, src, s=1.0, c=0.0):
            nc.scalar.activation(dst, src, AF.Sign, bias=c, scale=s)

        # persistent tensors
        x1 = pp.tile([64, NPTS], DT_F32, tag="x1")
        sx2 = pp.tile([64, NPTS], DT_BF, tag="sx2")
        maxsA = pp.tile([128, 8, BL, TPC], DT_F32, tag="maxsA")
        maxsB = pp.tile([128, 8, BL, TPC], DT_F32, tag="maxsB")
        maxsC = pp.tile([128, 8, BL, TPC], DT_F32, tag="maxsC")
        ones1 = pp.tile([1, 128], DT_F32, tag="ones1")
        nc.vector.memset(ones1[:], 1.0)

        def c3_block(wname, maxs, rhs, t, b):
            """1024-wide pooled layer: 8 matmul chunks (as 4 bank-pairs), raw max."""
            tc_i = t % TPC
            for mcp in range(4):
                pC = psp.tile([128, 2, TPTS], DT_F32, tag="ppair")
                for h in range(2):
                    mc = mcp * 2 + h
                    mm(pC[:, h, :], w(wname)[:, mc * 128:(mc + 1) * 128], rhs)
                dst = maxs[:, mcp * 2:(mcp + 1) * 2, b, tc_i]
                if (t + mcp) % 4 < DIRECT_PAIRS:
                    nc.vector.tensor_reduce(out=dst, in_=pC[:], axis=AX.X, op=ALU.max)
                else:
                    a3 = sb.tile([128, 2, TPTS], DT_BF, tag="a3")
                    nc.scalar.activation(a3[:], pC[:], AF.Copy)
                    nc.vector.tensor_reduce(out=dst, in_=a3[:], axis=AX.X, op=ALU.max)

        def pooled_sign(maxs, spfx, tag):
            """maxs [128,8,BL,TPC] -> pooled [128,8,BL] raw, sign(affine) bf16."""
            pooled = pp.tile([128, 8, BL], DT_F32, tag=tag + "_raw")
            nc.vector.tensor_reduce(out=pooled[:], in_=maxs[:], axis=AX.X, op=ALU.max)
            sp = pp.tile([128, 8, BL], DT_BF, tag=tag)
            for mc in range(8):
                sgn(sp[:, mc, :], pooled[:, mc, :],
                    w(spfx + "_s")[:, mc:mc + 1], w(spfx + "_c")[:, mc:mc + 1])
            return pooled, sp

        def stn_mlp(sp, pfx, fm3, f3ctag):
            """f1 -> f2 -> f3(+bias+eye) on pooled signs; returns f3 sbuf [128,fm3,BL]."""
            sf1 = sb.tile([128, 4, BL], DT_BF, tag=pfx + "sf1")
            for mc in range(4):
                pf = pss.tile([128, BL], DT_F32, tag="psmall")
                for kc in range(8):
                    mm(pf, w(pfx + "_f1_w")[:, kc, mc * 128:(mc + 1) * 128],
                       sp[:, kc, :], start=(kc == 0), stop=(kc == 7))
                sgn(sf1[:, mc, :], pf,
                    w(pfx + "f1_s")[:, mc:mc + 1], w(pfx + "f1_c")[:, mc:mc + 1])
            sf2 = sb.tile([128, 2, BL], DT_BF, tag=pfx + "sf2")
            for mc in range(2):
                pf = pss.tile([128, BL], DT_F32, tag="psmall")
                for kc in range(4):
                    mm(pf, w(pfx + "_f2_w")[:, kc, mc * 128:(mc + 1) * 128],
                       sf1[:, kc, :], start=(kc == 0), stop=(kc == 3))
                sgn(sf2[:, mc, :], pf,
                    w(pfx + "f2_s")[:, mc:mc + 1], w(pfx + "f2_c")[:, mc:mc + 1])
            mdim = 128 if fm3 > 1 else wmap[pfx + "_f3_w"].shape[2]
            out = sb.tile([mdim, fm3, BL], DT_F32, tag=f3ctag)
            for mc in range(fm3):
                pf = pss.tile([mdim, BL], DT_F32, tag="psmall")
                for kc in range(2):
                    mm(pf, w(pfx + "_f3_w")[:, kc, mc * 128:mc * 128 + mdim],
                       sf2[:, kc, :], start=(kc == 0), stop=(kc == 1))
                nc.vector.tensor_scalar(out[:, mc, :], pf,
                                        w(pfx + "_f3_c")[:, mc:mc + 1], None,
                                        op0=ALU.add)
            return out

        # ================= phase A: stn on sign(x) =================
        for t in range(NT):
            b = t // TPC
            col = bass.ds(t * TPTS, TPTS)
            sxt = sb.tile([9, TPTS], DT_BF, tag="sxt")
            nc.sync.dma_start(out=sxt[:], in_=sxT_d[:, col])
            pA = ps.tile([64, TPTS], DT_F32, tag="p512")
            mm(pA, w("stn_c1_w")[:], sxt[:])
            a1 = sb.tile([64, TPTS], DT_BF, tag="a1")
            sgn(a1, pA, w("stn1_s")[:], w("stn1_c")[:])
            pB = ps.tile([128, TPTS], DT_F32, tag="p512")
            mm(pB, w("stn_c2_w")[:], a1[:])
            a2 = sb.tile([128, TPTS], DT_BF, tag="a2")
            sgn(a2, pB, w("stn2_s")[:], w("stn2_c")[:])
            c3_block("stn_c3_w", maxsA, a2[:], t, b)

        _, spA = pooled_sign(maxsA, "stn3", "spA")
        trans_sb = stn_mlp(spA, "stn", 1, "trans_sb")  # [9, 1, BL]
        tsc = dr.tile([9, BL], DT_F32, tag="tsc")
        nc.sync.dma_start(out=tsc[:], in_=trans_sb[:, 0, :])
        nc.sync.dma_start(out=tr_d, in_=tsc[:].rearrange("(i j) b -> b i j", i=3))
        T3 = pp.tile([3, BL, 3], DT_F32, tag="T3")
        for b in range(BL):
            nc.sync.dma_start(out=T3[:, b, :],
                              in_=tsc[:, b].rearrange("(i j) -> i j", i=3))

        # ================= phase B: xyz transform, enc c1, fstn =================
        for t in range(NT):
            b = t // TPC
            col = bass.ds(t * TPTS, TPTS)
            xyzt = sb.tile([3, TPTS], DT_F32, tag="xyzt")
            nc.sync.dma_start(out=xyzt[:], in_=xyzT_d[:, col])
            sft = sb.tile([6, TPTS], DT_BF, tag="sft")
            nc.sync.dma_start(out=sft[:], in_=sfT_d[:, col])
            pXY = ps.tile([3, TPTS], DT_F32, tag="p512")
            mm(pXY, T3[:, b, :], xyzt[:])
            sxyz = sb.tile([3, TPTS], DT_BF, tag="sxyz")
            sgn(sxyz, pXY)
            pE1 = ps.tile([64, TPTS], DT_F32, tag="p512")
            mm(pE1, w("enc_c1a_w")[:], sxyz[:], start=True, stop=False)
            mm(pE1, w("enc_c1b_w")[:], sft[:], start=False, stop=True)
            sx1 = sb.tile([64, TPTS], DT_BF, tag="sx1")
            sgn(sx1, pE1, w("enc1_s")[:], w("enc1_c")[:])
            nc.scalar.activation(x1[:, col], pE1, AF.Identity,
                                 bias=w("enc1_c")[:], scale=w("enc1_s")[:])
            nc.vector.tensor_scalar(x1[:, col], x1[:, col], 1.0, -1.0,
                                    op0=ALU.min, op1=ALU.max)
            pF = ps.tile([64, TPTS], DT_F32, tag="p512")
            mm(pF, w("fstn_c1_w")[:], sx1[:])
            sfc1 = sb.tile([64, TPTS], DT_BF, tag="sfc1")
            sgn(sfc1, pF, w("fstn1_s")[:], w("fstn1_c")[:])
            pG = ps.tile([128, TPTS], DT_F32, tag="p512")
            mm(pG, w("fstn_c2_w")[:], sfc1[:])
            sfc2 = sb.tile([128, TPTS], DT_BF, tag="sfc2")
            sgn(sfc2, pG, w("fstn2_s")[:], w("fstn2_c")[:])
            c3_block("fstn_c3_w", maxsB, sfc2[:], t, b)

        _, spB = pooled_sign(maxsB, "fstn3", "spB")
        tfsb = stn_mlp(spB, "fstn", 32, "tfsb")  # [128, 32, BL]
        tfsc = dr.tile([32, 128, BL], DT_F32, tag="tfsc")
        nc.sync.dma_start(out=tfsc[:].rearrange("m p b -> p m b"), in_=tfsb[:])
        nc.sync.dma_start(out=tf_d.rearrange("b i j -> b (i j)"),
                          in_=tfsc[:].rearrange("m p b -> b (m p)"))
        T64 = pp.tile([64, BL, 64], DT_F32, tag="T64")
        for b in range(BL):
            nc.sync.dma_start(out=T64[:, b, :],
                              in_=tfsc[:, :, b].rearrange("m (h j) -> (m h) j", h=2))

        # ================= phase C: feature transform, enc c2/c3, g =================
        for t in range(NT):
            b = t // TPC
            col = bass.ds(t * TPTS, TPTS)
            pX2 = ps.tile([64, TPTS], DT_F32, tag="p512")
            mm(pX2, T64[:, b, :], x1[:, col])
            sgn(sx2[:, col], pX2)
            pH = ps.tile([128, TPTS], DT_F32, tag="p512")
            mm(pH, w("enc_c2_w")[:], sx2[:, col])
            sh = sb.tile([128, TPTS], DT_BF, tag="sh")
            sgn(sh, pH, w("enc2_s")[:], w("enc2_c")[:])
            c3_block("enc_c3_w", maxsC, sh[:], t, b)

        _, sgC = pooled_sign(maxsC, "enc3", "sgC")
        # head c1 global-feature contribution -> per-cloud bias
        kg = sb.tile([128, 4, BL], DT_F32, tag="kg")
        cb = pp.tile([128, 4, BL], DT_F32, tag="cb")
        for mc in range(4):
            pk = pss.tile([128, BL], DT_F32, tag="psmall")
            for kc in range(8):
                mm(pk, w("head_c1g_w")[:, kc, mc * 128:(mc + 1) * 128],
                   sgC[:, kc, :], start=(kc == 0), stop=(kc == 7))
            nc.scalar.copy(kg[:, mc, :], pk)
            nc.vector.tensor_scalar(cb[:, mc, :], kg[:, mc, :],
                                    w("head1_s")[:, mc:mc + 1],
                                    w("head1_c")[:, mc:mc + 1],
                                    op0=ALU.mult, op1=ALU.add)

        # ================= phase D: seg head + log_softmax =================
        for t in range(NT):
            b = t // TPC
            col = bass.ds(t * TPTS, TPTS)
            sd1 = sb.tile([128, 4, TPTS], DT_BF, tag="sd1")
            for mc in range(4):
                pD = ps.tile([128, TPTS], DT_F32, tag="p512")
                mm(pD, w("head_c1p_w")[:, mc * 128:(mc + 1) * 128], sx2[:, col])
                nc.scalar.activation(sd1[:, mc, :], pD, AF.Sign,
                                     bias=cb[:, mc, b:b + 1],
                                     scale=w("head1_s")[:, mc:mc + 1])
            se = sb.tile([128, 2, TPTS], DT_BF, tag="se")
            for mc in range(2):
                pE = ps.tile([128, TPTS], DT_F32, tag="p512")
                for kc in range(4):
                    mm(pE, w("head_c2_w")[:, kc, mc * 128:(mc + 1) * 128],
                       sd1[:, kc, :], start=(kc == 0), stop=(kc == 3))
                sgn(se[:, mc, :], pE,
                    w("head2_s")[:, mc:mc + 1], w("head2_c")[:, mc:mc + 1])
            pF3 = ps.tile([128, TPTS], DT_F32, tag="p512")
            for kc in range(2):
                mm(pF3, w("head_c3_w")[:, kc, :], se[:, kc, :],
                   start=(kc == 0), stop=(kc == 1))
            h3 = sb.tile([128, TPTS], DT_F32, tag="h3")
            nc.scalar.activation(h3, pF3, AF.Identity,
                                 bias=w("head3_c")[:], scale=w("head3_s")[:])
            nc.vector.tensor_scalar(h3, h3, 1.0, -1.0, op0=ALU.min, op1=ALU.max)
            pL = pss.tile([128, 4, NUM_CLASS], DT_F32, tag="psmall")
            for pc in range(4):
                mm(pL[:, pc, :], h3[:, bass.ds(pc * 128, 128)], w("c4_wT")[:],
                   start=True, stop=False)
                mm(pL[:, pc, :], ones1[:], w("c4_b")[:], start=False, stop=True)
            ex = sb.tile([128, 4, NUM_CLASS], DT_F32, tag="ex")
            nc.scalar.activation(ex[:], pL[:], AF.Exp)
            ssum = sb.tile([128, 4], DT_F32, tag="ssum")
            nc.vector.tensor_reduce(out=ssum[:], in_=ex[:], axis=AX.X, op=ALU.add)
            lsum = sb.tile([128, 4], DT_F32, tag="lsum")
            nc.scalar.activation(lsum[:], ssum[:], AF.Ln)
            oT = sb.tile([128, 4, NUM_CLASS], DT_F32, tag="oT")
            for pc in range(4):
                nc.vector.tensor_scalar(oT[:, pc, :], pL[:, pc, :],
                                        lsum[:, pc:pc + 1], None, op0=ALU.subtract)
            nc.sync.dma_start(
                out=lo_d[bass.ds(t * TPTS, TPTS)].rearrange("(pc p) c -> p pc c", pc=4),
                in_=oT[:])

    nc.compile()
    return nc


# ---------------------------------------------------------------- entry point

_CACHE = {}


def _run(pos, feat, params, trace=False):
    pos = np.asarray(pos, F32)
    feat = np.asarray(feat, F32)
    wmap = _make_wmap(params)
    if "nc" not in _CACHE:
        _CACHE["nc"] = _build_program(wmap)
    nc = _CACHE["nc"]

    x = np.concatenate([pos, feat], axis=-1)               # (B, N, 9)
    sx = np.sign(x).astype(BF16)
    in_maps = []
    for i in range(M_CORES):
        sl = slice(i * BL, (i + 1) * BL)
        sxc = np.ascontiguousarray(sx[sl].transpose(2, 0, 1).reshape(9, NPTS))
        m = {k: v for k, v in wmap.items()}
        m["sxT"] = sxc
        m["sfT"] = np.ascontiguousarray(sxc[3:])
        m["xyzT"] = np.ascontiguousarray(
            pos[sl].transpose(2, 0, 1).reshape(3, NPTS).astype(F32))
        in_maps.append(m)

    res = run_bass_kernel_spmd(nc, in_maps, core_ids=list(range(M_CORES)),
                               trace=trace)
    out = np.concatenate([r["logout"] for r in res.results], axis=0)
    trans = np.concatenate([r["trans_o"] for r in res.results], axis=0)
    tf = np.concatenate([r["tf_o"] for r in res.results], axis=0)
    return (out, trans, tf), res


def kernel(pos, feat, params):
    (out, trans, tf), _ = _run(pos, feat, params, trace=False)
    return out, trans, tf


# revision 11
# speedup vs baseline: 1.0045x; 1.0045x over previous
"""Bass/Trainium2 kernel for BasicBiPointNetSemSeg (binarized PointNet semantic seg).

Data-parallel over 8 NeuronCores: batch 16 -> 2 point clouds per core.
Layout on device: channels on partitions, points on the free dim.

Key transformations (all exact, not approximations):
  - sign(W) precomputed on host, stored transposed as bf16 (+-1 exact in bf16).
  - sign(x) @ sign(W).T done as bf16 matmuls (integer accumulation, exact).
  - BatchNorm + bilinear bias folded into one affine (scale s>0, bias c) applied
    by the ScalarE activation: Sign(z*s + c) produces the next layer's +-1 input
    in one op.  ht (clip) before a sign is dropped: sign(clip(x)) == sign(x).
  - Max-pool layers (stn c3 / fstn c3 / enc c3): pool the RAW integer matmul
    outputs (monotone affine with s>0 commutes with max), apply the affine once
    per cloud after pooling.  Raw values are integers |z|<=128 -> bf16 exact.
  - Global feature g enters the seg head only via sign(g) @ Wg: that term is
    constant per cloud, computed once and folded into the head-c1 bias.

Scheduling: every layer is emitted as a sweep over all 16 point tiles with the
same stationary weight, so the PE gets dense same-weight matmul bursts (keeps
the HAM clock-gate warm and lets walrus LDWEIGHTS-dedup remove reloads).
"""

import os
import sys
from contextlib import ExitStack

import numpy as np
import ml_dtypes

for _p in ("/opt/trn_rl_repo",):
    if os.path.isdir(_p) and _p not in sys.path:
        sys.path.append(_p)

import concourse.bacc as bacc
import concourse.bass as bass
import concourse.tile as tile
from concourse import mybir
from concourse.bass_utils import run_bass_kernel_spmd
import concourse.bass_utils as _bu

BF16 = ml_dtypes.bfloat16
F32 = np.float32
DT_BF = mybir.dt.bfloat16
DT_F32 = mybir.dt.float32
AF = mybir.ActivationFunctionType
ALU = mybir.AluOpType
AX = mybir.AxisListType

B, N, NUM_CLASS = 16, 4096, 13
M_CORES = 8
BL = B // M_CORES          # clouds per core
NPTS = BL * N              # points per core
TPTS = 512                 # points per tile
NT = NPTS // TPTS          # tiles per core
TPC = N // TPTS            # tiles per cloud
EPS = 1e-5

# Of the 8 reduce slots per pooled layer (pattern index mod 8), how many are
# reduced by the VectorE straight from PSUM (rest: ScalarE bf16 copy + reduce).
ND8 = int(os.environ.get("ND8", "4"))

# Let walrus drop back-to-back redundant LDWEIGHTS (sweeps reuse the weight).
if os.environ.get("LDWOPT", "0") == "1" and not getattr(_bu, "_ldw_patched", False):
    _orig_run_command = _bu.run_command

    def _run_command_ldw(argv, **kw):
        argv = ["--enable-ldw-opt=true" if a == "--enable-ldw-opt=false" else a
                for a in argv]
        return _orig_run_command(argv, **kw)

    _bu.run_command = _run_command_ldw
    _bu._ldw_patched = True


# ---------------------------------------------------------------- host prep

def _sgnT(Wdict, kc=None):
    Wt = np.ascontiguousarray(np.sign(np.asarray(Wdict["W"], F32)).T.astype(BF16))
    if kc is not None:
        Wt = np.ascontiguousarray(Wt.reshape(kc, 128, -1))
    return Wt


def _fold(lin, bn):
    g = np.asarray(bn["g"], F32)
    v = np.asarray(bn["v"], F32)
    m = np.asarray(bn["m"], F32)
    be = np.asarray(bn["be"], F32)
    b = np.asarray(lin["b"], F32)
    s = g / np.sqrt(v + EPS)
    c = (b - m) * s + be
    return s.astype(F32), c.astype(F32)


def _chunked(vec, mc):
    """[M] -> [M,1] (mc==1) or [128, mc] with [p, j] = vec[j*128+p]."""
    vec = np.asarray(vec, F32)
    if mc == 1:
        return np.ascontiguousarray(vec.reshape(-1, 1))
    return np.ascontiguousarray(vec.reshape(mc, 128).T)


def _make_wmap(params):
    p = params
    feat = p["feat"]
    stn, fstn = feat["stn"], feat["fstn"]
    w = {}

    def affine(prefix, lin, bn, mc):
        s, c = _fold(lin, bn)
        w[prefix + "_s"] = _chunked(s, mc)
        w[prefix + "_c"] = _chunked(c, mc)

    # --- stn (k=3) ---
    w["stn_c1_w"] = _sgnT(stn["c1"])                 # [9, 64]
    affine("stn1", stn["c1"], stn["b1"], 1)
    w["stn_c2_w"] = _sgnT(stn["c2"])                 # [64, 128]
    affine("stn2", stn["c2"], stn["b2"], 1)
    w["stn_c3_w"] = _sgnT(stn["c3"])                 # [128, 1024]
    affine("stn3", stn["c3"], stn["b3"], 8)
    w["stn_f1_w"] = _sgnT(stn["f1"], kc=8)           # [8,128,512]
    affine("stnf1", stn["f1"], stn["b4"], 4)
    w["stn_f2_w"] = _sgnT(stn["f2"], kc=4)           # [4,128,256]
    affine("stnf2", stn["f2"], stn["b5"], 2)
    w["stn_f3_w"] = _sgnT(stn["f3"], kc=2)           # [2,128,9]
    w["stn_f3_c"] = _chunked(
        np.asarray(stn["f3"]["b"], F32) + np.eye(3, dtype=F32).reshape(-1), 1)

    # --- fstn (k=64) ---
    w["fstn_c1_w"] = _sgnT(fstn["c1"])               # [64, 64]
    affine("fstn1", fstn["c1"], fstn["b1"], 1)
    w["fstn_c2_w"] = _sgnT(fstn["c2"])               # [64, 128]
    affine("fstn2", fstn["c2"], fstn["b2"], 1)
    w["fstn_c3_w"] = _sgnT(fstn["c3"])               # [128, 1024]
    affine("fstn3", fstn["c3"], fstn["b3"], 8)
    w["fstn_f1_w"] = _sgnT(fstn["f1"], kc=8)
    affine("fstnf1", fstn["f1"], fstn["b4"], 4)
    w["fstn_f2_w"] = _sgnT(fstn["f2"], kc=4)
    affine("fstnf2", fstn["f2"], fstn["b5"], 2)
    w["fstn_f3_w"] = _sgnT(fstn["f3"], kc=2)         # [2,128,4096]
    w["fstn_f3_c"] = _chunked(
        np.asarray(fstn["f3"]["b"], F32) + np.eye(64, dtype=F32).reshape(-1), 32)

    # --- encoder ---
    w["enc_c1_w"] = _sgnT(feat["c1"])                # [9, 64]
    affine("enc1", feat["c1"], feat["b1"], 1)
    w["enc_c2_w"] = _sgnT(feat["c2"])                # [64, 128]
    affine("enc2", feat["c2"], feat["b2"], 1)
    w["enc_c3_w"] = _sgnT(feat["c3"])                # [128, 1024]
    affine("enc3", feat["c3"], feat["b3"], 8)

    # --- seg head ---
    c1W = np.sign(np.asarray(p["c1"]["W"], F32))     # [512, 1088]
    w["head_c1g_w"] = np.ascontiguousarray(
        c1W[:, :1024].T.astype(BF16).reshape(8, 128, 512))
    w["head_c1p_w"] = np.ascontiguousarray(c1W[:, 1024:].T.astype(BF16))  # [64,512]
    affine("head1", p["c1"], p["b1"], 4)
    w["head_c2_w"] = _sgnT(p["c2"], kc=4)            # [4,128,256]
    affine("head2", p["c2"], p["b2"], 2)
    w["head_c3_w"] = _sgnT(p["c3"], kc=2)            # [2,128,128]
    affine("head3", p["c3"], p["b3"], 1)
    w["c4_wT"] = np.ascontiguousarray(np.asarray(p["c4"]["W"], F32).T)    # [128,13]
    w["c4_b"] = np.ascontiguousarray(np.asarray(p["c4"]["b"], F32).reshape(1, 13))
    return w


# ---------------------------------------------------------------- device program

def _build_program(wmap):
    nc = bacc.Bacc("TRN2", target_bir_lowering=False, debug=False)
    dts = {}
    for name, arr in wmap.items():
        dt = DT_BF if arr.dtype == BF16 else DT_F32
        dts[name] = nc.dram_tensor(name, list(arr.shape), dt, kind="ExternalInput").ap()
    sxT_d = nc.dram_tensor("sxT", [9, NPTS], DT_BF, kind="ExternalInput").ap()
    xyzT_d = nc.dram_tensor("xyzT", [3, NPTS], DT_F32, kind="ExternalInput").ap()
    lo_d = nc.dram_tensor("logout", [NPTS, NUM_CLASS], DT_F32, kind="ExternalOutput").ap()
    tr_d = nc.dram_tensor("trans_o", [BL, 3, 3], DT_F32, kind="ExternalOutput").ap()
    tf_d = nc.dram_tensor("tf_o", [BL, 64, 64], DT_F32, kind="ExternalOutput").ap()

    def col(t):
        return bass.ds(t * TPTS, TPTS)

    with ExitStack() as ctx:
        tc = ctx.enter_context(tile.TileContext(nc))
        wp = ctx.enter_context(tc.tile_pool(name="wpool", bufs=1))
        pp = ctx.enter_context(tc.tile_pool(name="persist", bufs=1))
        sb = ctx.enter_context(tc.tile_pool(name="work", bufs=3))
        ps = ctx.enter_context(tc.tile_pool(name="psum", bufs=3, space="PSUM"))
        psp = ctx.enter_context(tc.tile_pool(name="psumpair", bufs=2, space="PSUM"))
        pss = ctx.enter_context(tc.tile_pool(name="psumsmall", bufs=1, space="PSUM"))
        dr = ctx.enter_context(tc.tile_pool(name="drsc", bufs=1, space="DRAM"))

        sbw = {}

        def w(name):
            if name not in sbw:
                ap = dts[name]
                arr = wmap[name]
                dt = DT_BF if arr.dtype == BF16 else DT_F32
                if arr.ndim == 3:  # [kc, 128, M] -> sbuf [128, kc, M]
                    kc = arr.shape[0]
                    t = wp.tile([128, kc, arr.shape[2]], dt, tag=name)
                    for k in range(kc):
                        nc.sync.dma_start(out=t[:, k, :], in_=ap[k])
                else:
                    t = wp.tile(list(arr.shape), dt, tag=name)
                    nc.sync.dma_start(out=t[:], in_=ap)
                sbw[name] = t
            return sbw[name]

        def mm(dst, lhsT, rhs, start=True, stop=True):
            nc.tensor.matmul(dst, lhsT, rhs, start=start, stop=stop)

        def sgn(dst, src, s=1.0, c=0.0):
            nc.scalar.activation(dst, src, AF.Sign, bias=c, scale=s)

        # persistent tensors
        s9 = pp.tile([9, NPTS], DT_BF, tag="s9")
        nc.sync.dma_start(out=s9[:], in_=sxT_d)
        # aliased buffers: same tag = same storage, disjoint lifetimes
        a1 = pp.tile([64, NPTS], DT_BF, tag="buf64a")        # phase A
        a2 = pp.tile([128, NPTS], DT_BF, tag="buf128")       # phase A
        x1 = pp.tile([64, NPTS], DT_F32, tag="bufx1")        # phase B -> C
        sx1 = pp.tile([64, NPTS], DT_BF, tag="buf64b")       # phase B
        sfc1 = pp.tile([64, NPTS], DT_BF, tag="buf64a")      # phase B (reuse a1)
        sfc2 = pp.tile([128, NPTS], DT_BF, tag="buf128")     # phase B (reuse a2)
        sx2 = pp.tile([64, NPTS], DT_BF, tag="buf64b")       # phase C -> D (reuse sx1)
        sh = pp.tile([128, NPTS], DT_BF, tag="buf128")       # phase C (reuse)
        maxsA = pp.tile([128, 8, BL, TPC], DT_F32, tag="maxsA")
        maxsB = pp.tile([128, 8, BL, TPC], DT_F32, tag="maxsB")
        maxsC = pp.tile([128, 8, BL, TPC], DT_F32, tag="maxsC")
        ones1 = pp.tile([1, 128], DT_F32, tag="ones1")
        nc.vector.memset(ones1[:], 1.0)

        def c3_sweep(wname, src_all, maxs):
            """1024-wide pooled layer: chunk-outer / tile-pair-inner, raw max."""
            for mc in range(8):
                lhs = w(wname)[:, mc * 128:(mc + 1) * 128]
                for tp in range(NT // 2):
                    t0 = 2 * tp
                    b = t0 // TPC
                    tc0 = t0 % TPC
                    pC = psp.tile([128, 2, TPTS], DT_F32, tag="ppair")
                    mm(pC[:, 0, :], lhs, src_all[:, col(t0)])
                    mm(pC[:, 1, :], lhs, src_all[:, col(t0 + 1)])
                    dst = maxs[:, mc, b, tc0:tc0 + 2]
                    if (mc * 8 + tp) % 8 < ND8:
                        nc.vector.tensor_reduce(out=dst, in_=pC[:], axis=AX.X,
                                                op=ALU.max)
                    else:
                        a3 = sb.tile([128, 2, TPTS], DT_BF, tag="a3")
                        nc.scalar.activation(a3[:], pC[:], AF.Copy)
                        nc.vector.tensor_reduce(out=dst, in_=a3[:], axis=AX.X,
                                                op=ALU.max)

        def pooled_sign(maxs, spfx, tag):
            pooled = pp.tile([128, 8, BL], DT_F32, tag=tag + "_raw")
            nc.vector.tensor_reduce(out=pooled[:], in_=maxs[:], axis=AX.X, op=ALU.max)
            sp = pp.tile([128, 8, BL], DT_BF, tag=tag)
            for mc in range(8):
                sgn(sp[:, mc, :], pooled[:, mc, :],
                    w(spfx + "_s")[:, mc:mc + 1], w(spfx + "_c")[:, mc:mc + 1])
            return pooled, sp

        def stn_mlp(sp, pfx, fm3, f3ctag):
            sf1 = sb.tile([128, 4, BL], DT_BF, tag=pfx + "sf1")
            for mc in range(4):
                pf = pss.tile([128, BL], DT_F32, tag="psmall")
                for kc in range(8):
                    mm(pf, w(pfx + "_f1_w")[:, kc, mc * 128:(mc + 1) * 128],
                       sp[:, kc, :], start=(kc == 0), stop=(kc == 7))
                sgn(sf1[:, mc, :], pf,
                    w(pfx + "f1_s")[:, mc:mc + 1], w(pfx + "f1_c")[:, mc:mc + 1])
            sf2 = sb.tile([128, 2, BL], DT_BF, tag=pfx + "sf2")
            for mc in range(2):
                pf = pss.tile([128, BL], DT_F32, tag="psmall")
                for kc in range(4):
                    mm(pf, w(pfx + "_f2_w")[:, kc, mc * 128:(mc + 1) * 128],
                       sf1[:, kc, :], start=(kc == 0), stop=(kc == 3))
                sgn(sf2[:, mc, :], pf,
                    w(pfx + "f2_s")[:, mc:mc + 1], w(pfx + "f2_c")[:, mc:mc + 1])
            mdim = 128 if fm3 > 1 else wmap[pfx + "_f3_w"].shape[2]
            out = sb.tile([mdim, fm3, BL], DT_F32, tag=f3ctag)
            for mc in range(fm3):
                pf = pss.tile([mdim, BL], DT_F32, tag="psmall")
                for kc in range(2):
                    mm(pf, w(pfx + "_f3_w")[:, kc, mc * 128:mc * 128 + mdim],
                       sf2[:, kc, :], start=(kc == 0), stop=(kc == 1))
                nc.vector.tensor_scalar(out[:, mc, :], pf,
                                        w(pfx + "_f3_c")[:, mc:mc + 1], None,
                                        op0=ALU.add)
            return out

        # ================= phase A: stn on sign(x) =================
        for t in range(NT):
            pA = ps.tile([64, TPTS], DT_F32, tag="p512")
            mm(pA, w("stn_c1_w")[:], s9[:, col(t)])
            sgn(a1[:, col(t)], pA, w("stn1_s")[:], w("stn1_c")[:])
        for t in range(NT):
            pB = ps.tile([128, TPTS], DT_F32, tag="p512")
            mm(pB, w("stn_c2_w")[:], a1[:, col(t)])
            sgn(a2[:, col(t)], pB, w("stn2_s")[:], w("stn2_c")[:])
        c3_sweep("stn_c3_w", a2, maxsA)

        _, spA = pooled_sign(maxsA, "stn3", "spA")
        trans_sb = stn_mlp(spA, "stn", 1, "trans_sb")  # [9, 1, BL]
        tsc = dr.tile([9, BL], DT_F32, tag="tsc")
        nc.sync.dma_start(out=tsc[:], in_=trans_sb[:, 0, :])
        nc.sync.dma_start(out=tr_d, in_=tsc[:].rearrange("(i j) b -> b i j", i=3))
        T3 = pp.tile([3, BL, 3], DT_F32, tag="T3")
        for b in range(BL):
            nc.sync.dma_start(out=T3[:, b, :],
                              in_=tsc[:, b].rearrange("(i j) -> i j", i=3))

        # ================= phase B: xyz transform, enc c1, fstn =================
        for t in range(NT):
            b = t // TPC
            xyzt = sb.tile([3, TPTS], DT_F32, tag="xyzt")
            nc.sync.dma_start(out=xyzt[:], in_=xyzT_d[:, col(t)])
            pXY = ps.tile([3, TPTS], DT_F32, tag="p512")
            mm(pXY, T3[:, b, :], xyzt[:])
            sgn(s9[0:3, col(t)], pXY)
        for t in range(NT):
            pE1 = ps.tile([64, TPTS], DT_F32, tag="p512")
            mm(pE1, w("enc_c1_w")[:], s9[:, col(t)])
            sgn(sx1[:, col(t)], pE1, w("enc1_s")[:], w("enc1_c")[:])
            nc.scalar.activation(x1[:, col(t)], pE1, AF.Identity,
                                 bias=w("enc1_c")[:], scale=w("enc1_s")[:])
            nc.vector.tensor_scalar(x1[:, col(t)], x1[:, col(t)], 1.0, -1.0,
                                    op0=ALU.min, op1=ALU.max)
        for t in range(NT):
            pF = ps.tile([64, TPTS], DT_F32, tag="p512")
            mm(pF, w("fstn_c1_w")[:], sx1[:, col(t)])
            sgn(sfc1[:, col(t)], pF, w("fstn1_s")[:], w("fstn1_c")[:])
        for t in range(NT):
            pG = ps.tile([128, TPTS], DT_F32, tag="p512")
            mm(pG, w("fstn_c2_w")[:], sfc1[:, col(t)])
            sgn(sfc2[:, col(t)], pG, w("fstn2_s")[:], w("fstn2_c")[:])
        c3_sweep("fstn_c3_w", sfc2, maxsB)

        _, spB = pooled_sign(maxsB, "fstn3", "spB")
        tfsb = stn_mlp(spB, "fstn", 32, "tfsb")  # [128, 32, BL]
        tfsc = dr.tile([32, 128, BL], DT_F32, tag="tfsc")
        nc.sync.dma_start(out=tfsc[:].rearrange("m p b -> p m b"), in_=tfsb[:])
        nc.sync.dma_start(out=tf_d.rearrange("b i j -> b (i j)"),
                          in_=tfsc[:].rearrange("m p b -> b (m p)"))
        T64 = pp.tile([64, BL, 64], DT_F32, tag="T64")
        for b in range(BL):
            nc.sync.dma_start(out=T64[:, b, :],
                              in_=tfsc[:, :, b].rearrange("m (h j) -> (m h) j", h=2))

        # ================= phase C: feature transform, enc c2/c3, g =================
        for t in range(NT):
            b = t // TPC
            pX2 = ps.tile([64, TPTS], DT_F32, tag="p512")
            mm(pX2, T64[:, b, :], x1[:, col(t)])
            sgn(sx2[:, col(t)], pX2)
        for t in range(NT):
            pH = ps.tile([128, TPTS], DT_F32, tag="p512")
            mm(pH, w("enc_c2_w")[:], sx2[:, col(t)])
            sgn(sh[:, col(t)], pH, w("enc2_s")[:], w("enc2_c")[:])
        c3_sweep("enc_c3_w", sh, maxsC)

        _, sgC = pooled_sign(maxsC, "enc3", "sgC")
        kg = sb.tile([128, 4, BL], DT_F32, tag="kg")
        cb = pp.tile([128, 4, BL], DT_F32, tag="cb")
        for mc in range(4):
            pk = pss.tile([128, BL], DT_F32, tag="psmall")
            for kc in range(8):
                mm(pk, w("head_c1g_w")[:, kc, mc * 128:(mc + 1) * 128],
                   sgC[:, kc, :], start=(kc == 0), stop=(kc == 7))
            nc.scalar.copy(kg[:, mc, :], pk)
            nc.vector.tensor_scalar(cb[:, mc, :], kg[:, mc, :],
                                    w("head1_s")[:, mc:mc + 1],
                                    w("head1_c")[:, mc:mc + 1],
                                    op0=ALU.mult, op1=ALU.add)

        # ================= phase D: seg head + log_softmax =================
        # two half-sweeps (8 tiles each) so the per-phase buffers stay small
        HALF = NT // 2
        for half in range(2):
            ts0 = half * HALF
            sd1 = pp.tile([128, 4, HALF * TPTS], DT_BF, tag="bufx1")
            se = pp.tile([128, 2, HALF * TPTS], DT_BF, tag="buf128")
            h3 = pp.tile([128, HALF * TPTS], DT_F32, tag="buf64a")

            def hcol(t):
                return bass.ds((t - ts0) * TPTS, TPTS)

            for mc in range(4):
                lhs = w("head_c1p_w")[:, mc * 128:(mc + 1) * 128]
                for t in range(ts0, ts0 + HALF):
                    b = t // TPC
                    pD = ps.tile([128, TPTS], DT_F32, tag="p512")
                    mm(pD, lhs, sx2[:, col(t)])
                    nc.scalar.activation(sd1[:, mc, hcol(t)], pD, AF.Sign,
                                         bias=cb[:, mc, b:b + 1],
                                         scale=w("head1_s")[:, mc:mc + 1])
            for mc in range(2):
                for t in range(ts0, ts0 + HALF):
                    pE = ps.tile([128, TPTS], DT_F32, tag="p512")
                    for kc in range(4):
                        mm(pE, w("head_c2_w")[:, kc, mc * 128:(mc + 1) * 128],
                           sd1[:, kc, hcol(t)], start=(kc == 0), stop=(kc == 3))
                    sgn(se[:, mc, hcol(t)], pE,
                        w("head2_s")[:, mc:mc + 1], w("head2_c")[:, mc:mc + 1])
            for t in range(ts0, ts0 + HALF):
                pF3 = ps.tile([128, TPTS], DT_F32, tag="p512")
                for kc in range(2):
                    mm(pF3, w("head_c3_w")[:, kc, :], se[:, kc, hcol(t)],
                       start=(kc == 0), stop=(kc == 1))
                nc.scalar.activation(h3[:, hcol(t)], pF3, AF.Identity,
                                     bias=w("head3_c")[:], scale=w("head3_s")[:])
                nc.vector.tensor_scalar(h3[:, hcol(t)], h3[:, hcol(t)], 1.0, -1.0,
                                        op0=ALU.min, op1=ALU.max)
            for t in range(ts0, ts0 + HALF):
                pL = pss.tile([128, 4, NUM_CLASS], DT_F32, tag="psmall")
                for pc in range(4):
                    mm(pL[:, pc, :],
                       h3[:, bass.ds((t - ts0) * TPTS + pc * 128, 128)],
                       w("c4_wT")[:], start=True, stop=False)
                    mm(pL[:, pc, :], ones1[:], w("c4_b")[:], start=False, stop=True)
                ex = sb.tile([128, 4, NUM_CLASS], DT_F32, tag="ex")
                nc.scalar.activation(ex[:], pL[:], AF.Exp)
                ssum = sb.tile([128, 4], DT_F32, tag="ssum")
                nc.vector.tensor_reduce(out=ssum[:], in_=ex[:], axis=AX.X, op=ALU.add)
                lsum = sb.tile([128, 4], DT_F32, tag="lsum")
                nc.scalar.activation(lsum[:], ssum[:], AF.Ln)
                oT = sb.tile([128, 4, NUM_CLASS], DT_F32, tag="oT")
                for pc in range(4):
                    nc.vector.tensor_scalar(oT[:, pc, :], pL[:, pc, :],
                                            lsum[:, pc:pc + 1], None,
                                            op0=ALU.subtract)
                nc.sync.dma_start(
                    out=lo_d[bass.ds(t * TPTS, TPTS)].rearrange(
                        "(pc p) c -> p pc c", pc=4),
                    in_=oT[:])

    nc.compile()
    return nc


# ---------------------------------------------------------------- entry point

_CACHE = {}


def _run(pos, feat, params, trace=False):
    pos = np.asarray(pos, F32)
    feat = np.asarray(feat, F32)
    wmap = _make_wmap(params)
    if "nc" not in _CACHE:
        _CACHE["nc"] = _build_program(wmap)
    nc = _CACHE["nc"]

    x = np.concatenate([pos, feat], axis=-1)               # (B, N, 9)
    sx = np.sign(x).astype(BF16)
    in_maps = []
    for i in range(M_CORES):
        sl = slice(i * BL, (i + 1) * BL)
        m = {k: v for k, v in wmap.items()}
        m["sxT"] = np.ascontiguousarray(sx[sl].transpose(2, 0, 1).reshape(9, NPTS))
        m["xyzT"] = np.ascontiguousarray(
            pos[sl].transpose(2, 0, 1).reshape(3, NPTS).astype(F32))
        in_maps.append(m)

    res = run_bass_kernel_spmd(nc, in_maps, core_ids=list(range(M_CORES)),
                               trace=trace)
    out = np.concatenate([r["logout"] for r in res.results], axis=0)
    trans = np.concatenate([r["trans_o"] for r in res.results], axis=0)
    tf = np.concatenate([r["tf_o"] for r in res.results], axis=0)
    return (out, trans, tf), res


def kernel(pos, feat, params):
    (out, trans, tf), _ = _run(pos, feat, params, trace=False)
    return out, trans, tf


# revision 12
# speedup vs baseline: 1.3513x; 1.3451x over previous
"""Bass/Trainium2 kernel for BasicBiPointNetSemSeg (binarized PointNet semantic seg).

Data-parallel over 8 NeuronCores: batch 16 -> 2 point clouds per core.
Layout on device: channels on partitions, points on the free dim.

Key transformations (all exact, not approximations):
  - sign(W) precomputed on host, stored transposed as bf16 (+-1 exact in bf16).
  - sign(x) @ sign(W).T done as bf16 matmuls (integer accumulation, exact).
  - BatchNorm + bilinear bias folded into one affine (scale s>0, bias c) applied
    by the ScalarE activation: Sign(z*s + c) produces the next layer's +-1 input
    in one op.  ht (clip) before a sign is dropped: sign(clip(x)) == sign(x).
  - Max-pool layers (stn c3 / fstn c3 / enc c3): pool the RAW integer matmul
    outputs (monotone affine with s>0 commutes with max), apply the affine once
    per cloud after pooling.  Raw values are integers |z|<=128 -> bf16 exact.
  - Global feature g enters the seg head only via sign(g) @ Wg: that term is
    constant per cloud, computed once and folded into the head-c1 bias.

Scheduling: every layer is emitted as a sweep over all 16 point tiles with the
same stationary weight, so the PE gets dense same-weight matmul bursts (keeps
the HAM clock-gate warm and lets walrus LDWEIGHTS-dedup remove reloads).
"""

import os
import sys
from contextlib import ExitStack

import numpy as np
import ml_dtypes

for _p in ("/opt/trn_rl_repo",):
    if os.path.isdir(_p) and _p not in sys.path:
        sys.path.append(_p)

import concourse.bacc as bacc
import concourse.bass as bass
import concourse.tile as tile
from concourse import mybir
from concourse.bass_utils import run_bass_kernel_spmd
import concourse.bass_utils as _bu

BF16 = ml_dtypes.bfloat16
F32 = np.float32
DT_BF = mybir.dt.bfloat16
DT_F32 = mybir.dt.float32
AF = mybir.ActivationFunctionType
ALU = mybir.AluOpType
AX = mybir.AxisListType

B, N, NUM_CLASS = 16, 4096, 13
M_CORES = 8
BL = B // M_CORES          # clouds per core
NPTS = BL * N              # points per core
TPTS = 512                 # points per tile
NT = NPTS // TPTS          # tiles per core
TPC = N // TPTS            # tiles per cloud
EPS = 1e-5

# Of the 8 reduce slots per pooled layer (pattern index mod 8), how many are
# reduced by the VectorE straight from PSUM (rest: ScalarE bf16 copy + reduce).
ND8 = int(os.environ.get("ND8", "4"))

# Let walrus drop back-to-back redundant LDWEIGHTS (sweeps reuse the weight).
if os.environ.get("LDWOPT", "0") == "1" and not getattr(_bu, "_ldw_patched", False):
    _orig_run_command = _bu.run_command

    def _run_command_ldw(argv, **kw):
        argv = ["--enable-ldw-opt=true" if a == "--enable-ldw-opt=false" else a
                for a in argv]
        return _orig_run_command(argv, **kw)

    _bu.run_command = _run_command_ldw
    _bu._ldw_patched = True


# ---------------------------------------------------------------- host prep

def _sgnT(Wdict, kc=None):
    Wt = np.ascontiguousarray(np.sign(np.asarray(Wdict["W"], F32)).T.astype(BF16))
    if kc is not None:
        Wt = np.ascontiguousarray(Wt.reshape(kc, 128, -1))
    return Wt


def _fold(lin, bn):
    g = np.asarray(bn["g"], F32)
    v = np.asarray(bn["v"], F32)
    m = np.asarray(bn["m"], F32)
    be = np.asarray(bn["be"], F32)
    b = np.asarray(lin["b"], F32)
    s = g / np.sqrt(v + EPS)
    c = (b - m) * s + be
    return s.astype(F32), c.astype(F32)


def _chunked(vec, mc):
    """[M] -> [M,1] (mc==1) or [128, mc] with [p, j] = vec[j*128+p]."""
    vec = np.asarray(vec, F32)
    if mc == 1:
        return np.ascontiguousarray(vec.reshape(-1, 1))
    return np.ascontiguousarray(vec.reshape(mc, 128).T)


def _make_wmap(params):
    p = params
    feat = p["feat"]
    stn, fstn = feat["stn"], feat["fstn"]
    w = {}

    def affine(prefix, lin, bn, mc):
        s, c = _fold(lin, bn)
        w[prefix + "_s"] = _chunked(s, mc)
        w[prefix + "_c"] = _chunked(c, mc)

    # --- stn (k=3) ---
    w["stn_c1_w"] = _sgnT(stn["c1"])                 # [9, 64]
    affine("stn1", stn["c1"], stn["b1"], 1)
    w["stn_c2_w"] = _sgnT(stn["c2"])                 # [64, 128]
    affine("stn2", stn["c2"], stn["b2"], 1)
    w["stn_c3_w"] = _sgnT(stn["c3"])                 # [128, 1024]
    affine("stn3", stn["c3"], stn["b3"], 8)
    w["stn_f1_w"] = _sgnT(stn["f1"], kc=8)           # [8,128,512]
    affine("stnf1", stn["f1"], stn["b4"], 4)
    w["stn_f2_w"] = _sgnT(stn["f2"], kc=4)           # [4,128,256]
    affine("stnf2", stn["f2"], stn["b5"], 2)
    # f3 output neurons permuted r=(i*k+j) -> r'=(j*k+i) so each M-chunk of
    # the matmul emits one transform column [i, b] directly (no reorder DMA).
    p3 = np.arange(9).reshape(3, 3).T.reshape(-1)
    w["stn_f3_w"] = np.ascontiguousarray(_sgnT(stn["f3"])[:, p3].reshape(2, 128, 9))
    c3v = (np.asarray(stn["f3"]["b"], F32) + np.eye(3, dtype=F32).reshape(-1))[p3]
    w["stn_f3_c"] = np.ascontiguousarray(c3v.reshape(3, 3).T)

    # --- fstn (k=64) ---
    w["fstn_c1_w"] = _sgnT(fstn["c1"])               # [64, 64]
    affine("fstn1", fstn["c1"], fstn["b1"], 1)
    w["fstn_c2_w"] = _sgnT(fstn["c2"])               # [64, 128]
    affine("fstn2", fstn["c2"], fstn["b2"], 1)
    w["fstn_c3_w"] = _sgnT(fstn["c3"])               # [128, 1024]
    affine("fstn3", fstn["c3"], fstn["b3"], 8)
    w["fstn_f1_w"] = _sgnT(fstn["f1"], kc=8)
    affine("fstnf1", fstn["f1"], fstn["b4"], 4)
    w["fstn_f2_w"] = _sgnT(fstn["f2"], kc=4)
    affine("fstnf2", fstn["f2"], fstn["b5"], 2)
    p64 = np.arange(4096).reshape(64, 64).T.reshape(-1)
    w["fstn_f3_w"] = np.ascontiguousarray(
        _sgnT(fstn["f3"])[:, p64].reshape(2, 128, 4096))
    c64v = (np.asarray(fstn["f3"]["b"], F32) + np.eye(64, dtype=F32).reshape(-1))[p64]
    w["fstn_f3_c"] = np.ascontiguousarray(c64v.reshape(64, 64).T)

    # --- encoder ---
    w["enc_c1_w"] = _sgnT(feat["c1"])                # [9, 64]
    affine("enc1", feat["c1"], feat["b1"], 1)
    w["enc_c2_w"] = _sgnT(feat["c2"])                # [64, 128]
    affine("enc2", feat["c2"], feat["b2"], 1)
    w["enc_c3_w"] = _sgnT(feat["c3"])                # [128, 1024]
    affine("enc3", feat["c3"], feat["b3"], 8)

    # --- seg head ---
    c1W = np.sign(np.asarray(p["c1"]["W"], F32))     # [512, 1088]
    w["head_c1g_w"] = np.ascontiguousarray(
        c1W[:, :1024].T.astype(BF16).reshape(8, 128, 512))
    w["head_c1p_w"] = np.ascontiguousarray(c1W[:, 1024:].T.astype(BF16))  # [64,512]
    affine("head1", p["c1"], p["b1"], 4)
    w["head_c2_w"] = _sgnT(p["c2"], kc=4)            # [4,128,256]
    affine("head2", p["c2"], p["b2"], 2)
    w["head_c3_w"] = _sgnT(p["c3"], kc=2)            # [2,128,128]
    affine("head3", p["c3"], p["b3"], 1)
    w["c4_wT"] = np.ascontiguousarray(np.asarray(p["c4"]["W"], F32).T)    # [128,13]
    w["c4_b"] = np.ascontiguousarray(np.asarray(p["c4"]["b"], F32).reshape(1, 13))
    return w


# ---------------------------------------------------------------- device program

def _build_program(wmap):
    nc = bacc.Bacc("TRN2", target_bir_lowering=False, debug=False)
    dts = {}
    for name, arr in wmap.items():
        dt = DT_BF if arr.dtype == BF16 else DT_F32
        dts[name] = nc.dram_tensor(name, list(arr.shape), dt, kind="ExternalInput").ap()
    sxT_d = nc.dram_tensor("sxT", [9, NPTS], DT_BF, kind="ExternalInput").ap()
    xyzT_d = nc.dram_tensor("xyzT", [3, NPTS], DT_F32, kind="ExternalInput").ap()
    lo_d = nc.dram_tensor("logout", [NPTS, NUM_CLASS], DT_F32, kind="ExternalOutput").ap()
    tr_d = nc.dram_tensor("trans_o", [BL, 3, 3], DT_F32, kind="ExternalOutput").ap()
    tf_d = nc.dram_tensor("tf_o", [BL, 64, 64], DT_F32, kind="ExternalOutput").ap()

    def col(t):
        return bass.ds(t * TPTS, TPTS)

    with ExitStack() as ctx:
        tc = ctx.enter_context(tile.TileContext(nc))
        wp = ctx.enter_context(tc.tile_pool(name="wpool", bufs=1))
        pp = ctx.enter_context(tc.tile_pool(name="persist", bufs=1))
        sb = ctx.enter_context(tc.tile_pool(name="work", bufs=3))
        ps = ctx.enter_context(tc.tile_pool(name="psum", bufs=3, space="PSUM"))
        psp = ctx.enter_context(tc.tile_pool(name="psumpair", bufs=2, space="PSUM"))
        pss = ctx.enter_context(tc.tile_pool(name="psumsmall", bufs=1, space="PSUM"))
        dr = ctx.enter_context(tc.tile_pool(name="drsc", bufs=1, space="DRAM"))

        sbw = {}

        def w(name):
            if name not in sbw:
                ap = dts[name]
                arr = wmap[name]
                dt = DT_BF if arr.dtype == BF16 else DT_F32
                if arr.ndim == 3:  # [kc, 128, M] -> sbuf [128, kc, M]
                    kc = arr.shape[0]
                    t = wp.tile([128, kc, arr.shape[2]], dt, tag=name)
                    for k in range(kc):
                        nc.sync.dma_start(out=t[:, k, :], in_=ap[k])
                else:
                    t = wp.tile(list(arr.shape), dt, tag=name)
                    nc.sync.dma_start(out=t[:], in_=ap)
                sbw[name] = t
            return sbw[name]

        def mm(dst, lhsT, rhs, start=True, stop=True):
            nc.tensor.matmul(dst, lhsT, rhs, start=start, stop=stop)

        def sgn(dst, src, s=1.0, c=0.0):
            nc.scalar.activation(dst, src, AF.Sign, bias=c, scale=s)

        # persistent tensors
        s9 = pp.tile([9, NPTS], DT_BF, tag="s9")
        nc.sync.dma_start(out=s9[:], in_=sxT_d)
        # aliased buffers: same tag = same storage, disjoint lifetimes
        a1 = pp.tile([64, NPTS], DT_BF, tag="buf64a")        # phase A
        a2 = pp.tile([128, NPTS], DT_BF, tag="buf128")       # phase A
        x1 = pp.tile([64, NPTS], DT_F32, tag="bufx1")        # phase B -> C
        sx1 = pp.tile([64, NPTS], DT_BF, tag="buf64b")       # phase B
        sfc1 = pp.tile([64, NPTS], DT_BF, tag="buf64a")      # phase B (reuse a1)
        sfc2 = pp.tile([128, NPTS], DT_BF, tag="buf128")     # phase B (reuse a2)
        sx2 = pp.tile([64, NPTS], DT_BF, tag="buf64b")       # phase C -> D (reuse sx1)
        sh = pp.tile([128, NPTS], DT_BF, tag="buf128")       # phase C (reuse)
        maxsA = pp.tile([128, 8, BL, TPC], DT_F32, tag="maxsA")
        maxsB = pp.tile([128, 8, BL, TPC], DT_F32, tag="maxsB")
        maxsC = pp.tile([128, 8, BL, TPC], DT_F32, tag="maxsC")
        ones1 = pp.tile([1, 128], DT_F32, tag="ones1")
        nc.vector.memset(ones1[:], 1.0)

        def c3_sweep(wname, src_all, maxs):
            """1024-wide pooled layer: chunk-outer / tile-pair-inner, raw max."""
            for mc in range(8):
                lhs = w(wname)[:, mc * 128:(mc + 1) * 128]
                for tp in range(NT // 2):
                    t0 = 2 * tp
                    b = t0 // TPC
                    tc0 = t0 % TPC
                    pC = psp.tile([128, 2, TPTS], DT_F32, tag="ppair")
                    mm(pC[:, 0, :], lhs, src_all[:, col(t0)])
                    mm(pC[:, 1, :], lhs, src_all[:, col(t0 + 1)])
                    dst = maxs[:, mc, b, tc0:tc0 + 2]
                    if (mc * 8 + tp) % 8 < ND8:
                        nc.vector.tensor_reduce(out=dst, in_=pC[:], axis=AX.X,
                                                op=ALU.max)
                    else:
                        a3 = sb.tile([128, 2, TPTS], DT_BF, tag="a3")
                        nc.scalar.activation(a3[:], pC[:], AF.Copy)
                        nc.vector.tensor_reduce(out=dst, in_=a3[:], axis=AX.X,
                                                op=ALU.max)

        def pooled_sign(maxs, spfx, tag):
            pooled = pp.tile([128, 8, BL], DT_F32, tag=tag + "_raw")
            nc.vector.tensor_reduce(out=pooled[:], in_=maxs[:], axis=AX.X, op=ALU.max)
            sp = pp.tile([128, 8, BL], DT_BF, tag=tag)
            for mc in range(8):
                sgn(sp[:, mc, :], pooled[:, mc, :],
                    w(spfx + "_s")[:, mc:mc + 1], w(spfx + "_c")[:, mc:mc + 1])
            return pooled, sp

        def stn_mlp(sp, pfx, fm3, f3ctag):
            sf1 = sb.tile([128, 4, BL], DT_BF, tag=pfx + "sf1")
            for mc in range(4):
                pf = ps.tile([128, BL], DT_F32, tag="p512")
                for kc in range(8):
                    mm(pf, w(pfx + "_f1_w")[:, kc, mc * 128:(mc + 1) * 128],
                       sp[:, kc, :], start=(kc == 0), stop=(kc == 7))
                sgn(sf1[:, mc, :], pf,
                    w(pfx + "f1_s")[:, mc:mc + 1], w(pfx + "f1_c")[:, mc:mc + 1])
            sf2 = sb.tile([128, 2, BL], DT_BF, tag=pfx + "sf2")
            for mc in range(2):
                pf = ps.tile([128, BL], DT_F32, tag="p512")
                for kc in range(4):
                    mm(pf, w(pfx + "_f2_w")[:, kc, mc * 128:(mc + 1) * 128],
                       sf1[:, kc, :], start=(kc == 0), stop=(kc == 3))
                sgn(sf2[:, mc, :], pf,
                    w(pfx + "f2_s")[:, mc:mc + 1], w(pfx + "f2_c")[:, mc:mc + 1])
            jdim = 3 if pfx == "stn" else 64
            out = pp.tile([jdim, BL, jdim], DT_F32, tag=f3ctag)  # [i, b, j]
            for jc in range(jdim):
                pf = ps.tile([jdim, BL], DT_F32, tag="p512")
                for kc in range(2):
                    mm(pf, w(pfx + "_f3_w")[:, kc, jc * jdim:(jc + 1) * jdim],
                       sf2[:, kc, :], start=(kc == 0), stop=(kc == 1))
                nc.vector.tensor_scalar(out[:, :, jc], pf,
                                        w(pfx + "_f3_c")[:, jc:jc + 1], None,
                                        op0=ALU.add)
            return out

        # ================= phase A: stn on sign(x) =================
        for t in range(NT):
            pA = ps.tile([64, TPTS], DT_F32, tag="p512")
            mm(pA, w("stn_c1_w")[:], s9[:, col(t)])
            sgn(a1[:, col(t)], pA, w("stn1_s")[:], w("stn1_c")[:])
        for t in range(NT):
            pB = ps.tile([128, TPTS], DT_F32, tag="p512")
            mm(pB, w("stn_c2_w")[:], a1[:, col(t)])
            sgn(a2[:, col(t)], pB, w("stn2_s")[:], w("stn2_c")[:])
        c3_sweep("stn_c3_w", a2, maxsA)

        _, spA = pooled_sign(maxsA, "stn3", "spA")
        T3 = stn_mlp(spA, "stn", 1, "T3")  # [3, b, 3] = trans[i, b, j]
        for b in range(BL):
            nc.sync.dma_start(out=tr_d[b], in_=T3[:, b, :])

        # ================= phase B: xyz transform, enc c1, fstn =================
        for t in range(NT):
            b = t // TPC
            xyzt = sb.tile([3, TPTS], DT_F32, tag="xyzt")
            nc.sync.dma_start(out=xyzt[:], in_=xyzT_d[:, col(t)])
            pXY = ps.tile([3, TPTS], DT_F32, tag="p512")
            mm(pXY, T3[:, b, :], xyzt[:])
            sgn(s9[0:3, col(t)], pXY)
        for t in range(NT):
            pE1 = ps.tile([64, TPTS], DT_F32, tag="p512")
            mm(pE1, w("enc_c1_w")[:], s9[:, col(t)])
            sgn(sx1[:, col(t)], pE1, w("enc1_s")[:], w("enc1_c")[:])
            nc.scalar.activation(x1[:, col(t)], pE1, AF.Identity,
                                 bias=w("enc1_c")[:], scale=w("enc1_s")[:])
            nc.vector.tensor_scalar(x1[:, col(t)], x1[:, col(t)], 1.0, -1.0,
                                    op0=ALU.min, op1=ALU.max)
        for t in range(NT):
            pF = ps.tile([64, TPTS], DT_F32, tag="p512")
            mm(pF, w("fstn_c1_w")[:], sx1[:, col(t)])
            sgn(sfc1[:, col(t)], pF, w("fstn1_s")[:], w("fstn1_c")[:])
        for t in range(NT):
            pG = ps.tile([128, TPTS], DT_F32, tag="p512")
            mm(pG, w("fstn_c2_w")[:], sfc1[:, col(t)])
            sgn(sfc2[:, col(t)], pG, w("fstn2_s")[:], w("fstn2_c")[:])
        c3_sweep("fstn_c3_w", sfc2, maxsB)

        _, spB = pooled_sign(maxsB, "fstn3", "spB")
        T64 = stn_mlp(spB, "fstn", 32, "T64")  # [64, b, 64] = trans_feat[i, b, j]
        for b in range(BL):
            nc.sync.dma_start(out=tf_d[b], in_=T64[:, b, :])

        # ================= phase C: feature transform, enc c2/c3, g =================
        for t in range(NT):
            b = t // TPC
            pX2 = ps.tile([64, TPTS], DT_F32, tag="p512")
            mm(pX2, T64[:, b, :], x1[:, col(t)])
            sgn(sx2[:, col(t)], pX2)
        for t in range(NT):
            pH = ps.tile([128, TPTS], DT_F32, tag="p512")
            mm(pH, w("enc_c2_w")[:], sx2[:, col(t)])
            sgn(sh[:, col(t)], pH, w("enc2_s")[:], w("enc2_c")[:])
        c3_sweep("enc_c3_w", sh, maxsC)

        _, sgC = pooled_sign(maxsC, "enc3", "sgC")
        kg = sb.tile([128, 4, BL], DT_F32, tag="kg")
        cb = pp.tile([128, 4, BL], DT_F32, tag="cb")
        for mc in range(4):
            pk = ps.tile([128, BL], DT_F32, tag="p512")
            for kc in range(8):
                mm(pk, w("head_c1g_w")[:, kc, mc * 128:(mc + 1) * 128],
                   sgC[:, kc, :], start=(kc == 0), stop=(kc == 7))
            nc.scalar.copy(kg[:, mc, :], pk)
            nc.vector.tensor_scalar(cb[:, mc, :], kg[:, mc, :],
                                    w("head1_s")[:, mc:mc + 1],
                                    w("head1_c")[:, mc:mc + 1],
                                    op0=ALU.mult, op1=ALU.add)

        # ================= phase D: seg head + log_softmax =================
        # two half-sweeps (8 tiles each) so the per-phase buffers stay small
        HALF = NT // 2
        for half in range(2):
            ts0 = half * HALF
            sd1 = pp.tile([128, 4, HALF * TPTS], DT_BF, tag="bufx1")
            se = pp.tile([128, 2, HALF * TPTS], DT_BF, tag="buf128")
            h3 = pp.tile([128, HALF * TPTS], DT_F32, tag="buf64a")

            def hcol(t):
                return bass.ds((t - ts0) * TPTS, TPTS)

            for mc in range(4):
                lhs = w("head_c1p_w")[:, mc * 128:(mc + 1) * 128]
                for t in range(ts0, ts0 + HALF):
                    b = t // TPC
                    pD = ps.tile([128, TPTS], DT_F32, tag="p512")
                    mm(pD, lhs, sx2[:, col(t)])
                    nc.scalar.activation(sd1[:, mc, hcol(t)], pD, AF.Sign,
                                         bias=cb[:, mc, b:b + 1],
                                         scale=w("head1_s")[:, mc:mc + 1])
            for mc in range(2):
                for t in range(ts0, ts0 + HALF):
                    pE = ps.tile([128, TPTS], DT_F32, tag="p512")
                    for kc in range(4):
                        mm(pE, w("head_c2_w")[:, kc, mc * 128:(mc + 1) * 128],
                           sd1[:, kc, hcol(t)], start=(kc == 0), stop=(kc == 3))
                    sgn(se[:, mc, hcol(t)], pE,
                        w("head2_s")[:, mc:mc + 1], w("head2_c")[:, mc:mc + 1])
            for t in range(ts0, ts0 + HALF):
                pF3 = ps.tile([128, TPTS], DT_F32, tag="p512")
                for kc in range(2):
                    mm(pF3, w("head_c3_w")[:, kc, :], se[:, kc, hcol(t)],
                       start=(kc == 0), stop=(kc == 1))
                nc.scalar.activation(h3[:, hcol(t)], pF3, AF.Identity,
                                     bias=w("head3_c")[:], scale=w("head3_s")[:])
                nc.vector.tensor_scalar(h3[:, hcol(t)], h3[:, hcol(t)], 1.0, -1.0,
                                        op0=ALU.min, op1=ALU.max)
            for t in range(ts0, ts0 + HALF):
                pL = pss.tile([128, 4, NUM_CLASS], DT_F32, tag="psmall")
                for pc in range(4):
                    mm(pL[:, pc, :],
                       h3[:, bass.ds((t - ts0) * TPTS + pc * 128, 128)],
                       w("c4_wT")[:], start=True, stop=False)
                    mm(pL[:, pc, :], ones1[:], w("c4_b")[:], start=False, stop=True)
                ex = sb.tile([128, 4, NUM_CLASS], DT_F32, tag="ex")
                nc.scalar.activation(ex[:], pL[:], AF.Exp)
                ssum = sb.tile([128, 4], DT_F32, tag="ssum")
                nc.vector.tensor_reduce(out=ssum[:], in_=ex[:], axis=AX.X, op=ALU.add)
                lsum = sb.tile([128, 4], DT_F32, tag="lsum")
                nc.scalar.activation(lsum[:], ssum[:], AF.Ln)
                oT = sb.tile([128, 4, NUM_CLASS], DT_F32, tag="oT")
                for pc in range(4):
                    nc.vector.tensor_scalar(oT[:, pc, :], pL[:, pc, :],
                                            lsum[:, pc:pc + 1], None,
                                            op0=ALU.subtract)
                nc.sync.dma_start(
                    out=lo_d[bass.ds(t * TPTS, TPTS)].rearrange(
                        "(pc p) c -> p pc c", pc=4),
                    in_=oT[:])

    nc.compile()
    return nc


# ---------------------------------------------------------------- entry point

_CACHE = {}


def _run(pos, feat, params, trace=False):
    pos = np.asarray(pos, F32)
    feat = np.asarray(feat, F32)
    wmap = _make_wmap(params)
    if "nc" not in _CACHE:
        _CACHE["nc"] = _build_program(wmap)
    nc = _CACHE["nc"]

    x = np.concatenate([pos, feat], axis=-1)               # (B, N, 9)
    sx = np.sign(x).astype(BF16)
    in_maps = []
    for i in range(M_CORES):
        sl = slice(i * BL, (i + 1) * BL)
        m = {k: v for k, v in wmap.items()}
        m["sxT"] = np.ascontiguousarray(sx[sl].transpose(2, 0, 1).reshape(9, NPTS))
        m["xyzT"] = np.ascontiguousarray(
            pos[sl].transpose(2, 0, 1).reshape(3, NPTS).astype(F32))
        in_maps.append(m)

    res = run_bass_kernel_spmd(nc, in_maps, core_ids=list(range(M_CORES)),
                               trace=trace)
    out = np.concatenate([r["logout"] for r in res.results], axis=0)
    trans = np.concatenate([r["trans_o"] for r in res.results], axis=0)
    tf = np.concatenate([r["tf_o"] for r in res.results], axis=0)
    return (out, trans, tf), res


def kernel(pos, feat, params):
    (out, trans, tf), _ = _run(pos, feat, params, trace=False)
    return out, trans, tf


# revision 15
# speedup vs baseline: 1.3927x; 1.0307x over previous
"""Bass/Trainium2 kernel for BasicBiPointNetSemSeg (binarized PointNet semantic seg).

Data-parallel over 8 NeuronCores: batch 16 -> 2 point clouds per core.
Layout on device: channels on partitions, points on the free dim.

Key transformations (all exact, not approximations):
  - sign(W) precomputed on host, stored transposed as bf16 (+-1 exact in bf16).
  - sign(x) @ sign(W).T done as bf16 matmuls (integer accumulation, exact).
  - BatchNorm + bilinear bias folded into one affine (scale s>0, bias c) applied
    by the ScalarE activation: Sign(z*s + c) produces the next layer's +-1 input
    in one op.  ht (clip) before a sign is dropped: sign(clip(x)) == sign(x).
  - Max-pool layers (stn c3 / fstn c3 / enc c3): pool the RAW integer matmul
    outputs (monotone affine with s>0 commutes with max), apply the affine once
    per cloud after pooling.  Raw values are integers |z|<=128 -> bf16 exact.
  - Global feature g enters the seg head only via sign(g) @ Wg: that term is
    constant per cloud, computed once and folded into the head-c1 bias.

Scheduling: every layer is emitted as a sweep over all 16 point tiles with the
same stationary weight, so the PE gets dense same-weight matmul bursts (keeps
the HAM clock-gate warm and lets walrus LDWEIGHTS-dedup remove reloads).
"""

import os
import sys
from contextlib import ExitStack

import numpy as np
import ml_dtypes

for _p in ("/opt/trn_rl_repo",):
    if os.path.isdir(_p) and _p not in sys.path:
        sys.path.append(_p)

import concourse.bacc as bacc
import concourse.bass as bass
import concourse.tile as tile
from concourse import mybir
from concourse.bass_utils import run_bass_kernel_spmd
import concourse.bass_utils as _bu

BF16 = ml_dtypes.bfloat16
F32 = np.float32
DT_BF = mybir.dt.bfloat16
DT_F32 = mybir.dt.float32
AF = mybir.ActivationFunctionType
ALU = mybir.AluOpType
AX = mybir.AxisListType

B, N, NUM_CLASS = 16, 4096, 13
M_CORES = 8
BL = B // M_CORES          # clouds per core
NPTS = BL * N              # points per core
TPTS = 512                 # points per tile
NT = NPTS // TPTS          # tiles per core
TPC = N // TPTS            # tiles per cloud
EPS = 1e-5

# Of the 8 reduce slots per pooled layer (pattern index mod 8), how many are
# reduced by the VectorE straight from PSUM (rest: ScalarE bf16 copy + reduce).
ND8 = int(os.environ.get("ND8", "4"))

# Let walrus drop back-to-back redundant LDWEIGHTS (sweeps reuse the weight).
if os.environ.get("LDWOPT", "0") == "1" and not getattr(_bu, "_ldw_patched", False):
    _orig_run_command = _bu.run_command

    def _run_command_ldw(argv, **kw):
        argv = ["--enable-ldw-opt=true" if a == "--enable-ldw-opt=false" else a
                for a in argv]
        return _orig_run_command(argv, **kw)

    _bu.run_command = _run_command_ldw
    _bu._ldw_patched = True


# ---------------------------------------------------------------- host prep

def _sgnT(Wdict, kc=None):
    Wt = np.ascontiguousarray(np.sign(np.asarray(Wdict["W"], F32)).T.astype(BF16))
    if kc is not None:
        Wt = np.ascontiguousarray(Wt.reshape(kc, 128, -1))
    return Wt


def _fold(lin, bn):
    g = np.asarray(bn["g"], F32)
    v = np.asarray(bn["v"], F32)
    m = np.asarray(bn["m"], F32)
    be = np.asarray(bn["be"], F32)
    b = np.asarray(lin["b"], F32)
    s = g / np.sqrt(v + EPS)
    c = (b - m) * s + be
    return s.astype(F32), c.astype(F32)


def _chunked(vec, mc):
    """[M] -> [M,1] (mc==1) or [128, mc] with [p, j] = vec[j*128+p]."""
    vec = np.asarray(vec, F32)
    if mc == 1:
        return np.ascontiguousarray(vec.reshape(-1, 1))
    return np.ascontiguousarray(vec.reshape(mc, 128).T)


def _make_wmap(params):
    p = params
    feat = p["feat"]
    stn, fstn = feat["stn"], feat["fstn"]
    w = {}

    def affine(prefix, lin, bn, mc):
        s, c = _fold(lin, bn)
        w[prefix + "_s"] = _chunked(s, mc)
        w[prefix + "_c"] = _chunked(c, mc)

    # --- stn (k=3) ---
    w["stn_c1_w"] = _sgnT(stn["c1"])                 # [9, 64]
    affine("stn1", stn["c1"], stn["b1"], 1)
    w["stn_c2_w"] = _sgnT(stn["c2"])                 # [64, 128]
    affine("stn2", stn["c2"], stn["b2"], 1)
    w["stn_c3_w"] = _sgnT(stn["c3"])                 # [128, 1024]
    affine("stn3", stn["c3"], stn["b3"], 8)
    w["stn_f1_w"] = _sgnT(stn["f1"], kc=8)           # [8,128,512]
    affine("stnf1", stn["f1"], stn["b4"], 4)
    w["stn_f2_w"] = _sgnT(stn["f2"], kc=4)           # [4,128,256]
    affine("stnf2", stn["f2"], stn["b5"], 2)
    # f3 output neurons permuted r=(i*k+j) -> r'=(j*k+i) so each M-chunk of
    # the matmul emits one transform column [i, b] directly (no reorder DMA).
    p3 = np.arange(9).reshape(3, 3).T.reshape(-1)
    w["stn_f3_w"] = np.ascontiguousarray(_sgnT(stn["f3"])[:, p3].reshape(2, 128, 9))
    c3v = (np.asarray(stn["f3"]["b"], F32) + np.eye(3, dtype=F32).reshape(-1))[p3]
    w["stn_f3_c"] = np.ascontiguousarray(c3v.reshape(3, 3).T)

    # --- fstn (k=64) ---
    w["fstn_c1_w"] = _sgnT(fstn["c1"])               # [64, 64]
    affine("fstn1", fstn["c1"], fstn["b1"], 1)
    w["fstn_c2_w"] = _sgnT(fstn["c2"])               # [64, 128]
    affine("fstn2", fstn["c2"], fstn["b2"], 1)
    w["fstn_c3_w"] = _sgnT(fstn["c3"])               # [128, 1024]
    affine("fstn3", fstn["c3"], fstn["b3"], 8)
    w["fstn_f1_w"] = _sgnT(fstn["f1"], kc=8)
    affine("fstnf1", fstn["f1"], fstn["b4"], 4)
    w["fstn_f2_w"] = _sgnT(fstn["f2"], kc=4)
    affine("fstnf2", fstn["f2"], fstn["b5"], 2)
    p64 = np.arange(4096).reshape(64, 64).T.reshape(-1)
    w["fstn_f3_w"] = np.ascontiguousarray(
        _sgnT(fstn["f3"])[:, p64].reshape(2, 128, 4096))
    c64v = (np.asarray(fstn["f3"]["b"], F32) + np.eye(64, dtype=F32).reshape(-1))[p64]
    w["fstn_f3_c"] = np.ascontiguousarray(c64v.reshape(64, 64).T)

    # --- encoder ---
    w["enc_c1_w"] = _sgnT(feat["c1"])                # [9, 64]
    affine("enc1", feat["c1"], feat["b1"], 1)
    w["enc_c2_w"] = _sgnT(feat["c2"])                # [64, 128]
    affine("enc2", feat["c2"], feat["b2"], 1)
    w["enc_c3_w"] = _sgnT(feat["c3"])                # [128, 1024]
    affine("enc3", feat["c3"], feat["b3"], 8)

    # --- seg head ---
    c1W = np.sign(np.asarray(p["c1"]["W"], F32))     # [512, 1088]
    w["head_c1g_w"] = np.ascontiguousarray(
        c1W[:, :1024].T.astype(BF16).reshape(8, 128, 512))
    w["head_c1p_w"] = np.ascontiguousarray(c1W[:, 1024:].T.astype(BF16))  # [64,512]
    affine("head1", p["c1"], p["b1"], 4)
    w["head_c2_w"] = _sgnT(p["c2"], kc=4)            # [4,128,256]
    affine("head2", p["c2"], p["b2"], 2)
    w["head_c3_w"] = _sgnT(p["c3"], kc=2)            # [2,128,128]
    affine("head3", p["c3"], p["b3"], 1)
    w["c4_wT"] = np.ascontiguousarray(np.asarray(p["c4"]["W"], F32).T)    # [128,13]
    w["c4_b"] = np.ascontiguousarray(
        np.tile(np.asarray(p["c4"]["b"], F32), 4).reshape(1, 52))
    return w


# ---------------------------------------------------------------- device program

def _build_program(wmap):
    nc = bacc.Bacc("TRN2", target_bir_lowering=False, debug=False)
    dts = {}
    for name, arr in wmap.items():
        dt = DT_BF if arr.dtype == BF16 else DT_F32
        dts[name] = nc.dram_tensor(name, list(arr.shape), dt, kind="ExternalInput").ap()
    sxT_d = nc.dram_tensor("sxT", [9, NPTS], DT_BF, kind="ExternalInput").ap()
    xyzT_d = nc.dram_tensor("xyzT", [3, NPTS], DT_F32, kind="ExternalInput").ap()
    lo_d = nc.dram_tensor("logout", [NPTS, NUM_CLASS], DT_F32, kind="ExternalOutput").ap()
    tr_d = nc.dram_tensor("trans_o", [BL, 3, 3], DT_F32, kind="ExternalOutput").ap()
    tf_d = nc.dram_tensor("tf_o", [BL, 64, 64], DT_F32, kind="ExternalOutput").ap()

    def col(t):
        return bass.ds(t * TPTS, TPTS)

    TP2 = 2 * TPTS          # 1024-point pair tiles (bf16 PSUM, one bank)
    NU = NPTS // TP2        # 8 pair tiles per core

    def col2(u):
        return bass.ds(u * TP2, TP2)

    with ExitStack() as ctx:
        tc = ctx.enter_context(tile.TileContext(nc))
        wp = ctx.enter_context(tc.tile_pool(name="wpool", bufs=1))
        pp = ctx.enter_context(tc.tile_pool(name="persist", bufs=1))
        sb = ctx.enter_context(tc.tile_pool(name="work", bufs=3))
        ps = ctx.enter_context(tc.tile_pool(name="psum", bufs=3, space="PSUM"))
        psb = ctx.enter_context(tc.tile_pool(name="psumb", bufs=2, space="PSUM"))
        pss = ctx.enter_context(tc.tile_pool(name="psumsmall", bufs=1, space="PSUM"))

        sbw = {}

        def w(name):
            if name not in sbw:
                ap = dts[name]
                arr = wmap[name]
                dt = DT_BF if arr.dtype == BF16 else DT_F32
                if arr.ndim == 3:  # [kc, 128, M] -> sbuf [128, kc, M]
                    kc = arr.shape[0]
                    t = wp.tile([128, kc, arr.shape[2]], dt, tag=name)
                    for k in range(kc):
                        nc.sync.dma_start(out=t[:, k, :], in_=ap[k])
                else:
                    t = wp.tile(list(arr.shape), dt, tag=name)
                    nc.sync.dma_start(out=t[:], in_=ap)
                sbw[name] = t
            return sbw[name]

        def mm(dst, lhsT, rhs, start=True, stop=True):
            nc.tensor.matmul(dst, lhsT, rhs, start=start, stop=stop)

        def sgn(dst, src, s=1.0, c=0.0):
            nc.scalar.activation(dst, src, AF.Sign, bias=c, scale=s)

        # persistent tensors
        s9 = pp.tile([9, NPTS], DT_BF, tag="s9")
        nc.sync.dma_start(out=s9[:], in_=sxT_d)
        # aliased buffers: same tag = same storage, disjoint lifetimes
        a1 = pp.tile([64, NPTS], DT_BF, tag="buf64a")        # phase A
        a2 = pp.tile([128, NPTS], DT_BF, tag="buf128")       # phase A
        x1 = pp.tile([64, NPTS], DT_F32, tag="bufx1")        # phase B -> C
        sx1 = pp.tile([64, NPTS], DT_BF, tag="buf64b")       # phase B
        sfc1 = pp.tile([64, NPTS], DT_BF, tag="buf64a")      # phase B (reuse a1)
        sfc2 = pp.tile([128, NPTS], DT_BF, tag="buf128")     # phase B (reuse a2)
        sx2 = pp.tile([64, NPTS], DT_BF, tag="buf64b")       # phase C -> D
        sh = pp.tile([128, NPTS], DT_BF, tag="buf128")       # phase C (reuse)
        maxsA = pp.tile([128, 8, BL, TPC], DT_F32, tag="maxsA")
        maxsB = pp.tile([128, 8, BL, TPC], DT_F32, tag="maxsB")
        maxsC = pp.tile([128, 8, BL, TPC], DT_F32, tag="maxsC")
        ones1 = pp.tile([1, 128], DT_F32, tag="ones1")
        nc.vector.memset(ones1[:], 1.0)

        def sweep1024(wname, spfx, src_all, dst_all):
            """K<=128 sign layer over 1024-point pair tiles (2-bank fp32 PSUM,
            one wide Sign per pair)."""
            lhs = w(wname)[:]
            md = lhs.shape[-1]
            s_ap = w(spfx + "_s")[:]
            c_ap = w(spfx + "_c")[:]
            for u in range(NU):
                pT = psb.tile([md, 2, TPTS], DT_F32, tag="pb1024")
                mm(pT[:, 0, :], lhs, src_all[:, col(2 * u)])
                mm(pT[:, 1, :], lhs, src_all[:, col(2 * u + 1)])
                sgn(dst_all[:, col2(u)], pT[:].rearrange("p h n -> p (h n)"),
                    s_ap, c_ap)

        def c3_sweep(wname, src_all, maxs):
            """1024-wide pooled layer: chunk-outer / pair-tile-inner, raw max."""
            for mc in range(8):
                lhs = w(wname)[:, mc * 128:(mc + 1) * 128]
                for u in range(NU):
                    b = u // (NU // BL)
                    tc0 = (2 * u) % TPC
                    pC = psb.tile([128, 2, TPTS], DT_F32, tag="pb1024")
                    mm(pC[:, 0, :], lhs, src_all[:, col(2 * u)])
                    mm(pC[:, 1, :], lhs, src_all[:, col(2 * u + 1)])
                    dst = maxs[:, mc, b, tc0:tc0 + 2]
                    if (mc * NU + u) % 8 < ND8:
                        nc.vector.tensor_reduce(out=dst, in_=pC[:], axis=AX.X,
                                                op=ALU.max)
                    else:
                        a3 = sb.tile([128, 2, TPTS], DT_BF, tag="a3")
                        nc.scalar.activation(a3[:], pC[:], AF.Copy)
                        nc.vector.tensor_reduce(out=dst, in_=a3[:], axis=AX.X,
                                                op=ALU.max)

        def pooled_sign(maxs, spfx, tag):
            pooled = pp.tile([128, 8, BL], DT_F32, tag=tag + "_raw")
            nc.vector.tensor_reduce(out=pooled[:], in_=maxs[:], axis=AX.X, op=ALU.max)
            sp = pp.tile([128, 8, BL], DT_BF, tag=tag)
            for mc in range(8):
                sgn(sp[:, mc, :], pooled[:, mc, :],
                    w(spfx + "_s")[:, mc:mc + 1], w(spfx + "_c")[:, mc:mc + 1])
            return pooled, sp

        def stn_mlp(sp, pfx, f3ctag):
            sf1 = sb.tile([128, 4, BL], DT_BF, tag=pfx + "sf1")
            for mc in range(4):
                pf = ps.tile([128, BL], DT_F32, tag="p512")
                for kc in range(8):
                    mm(pf, w(pfx + "_f1_w")[:, kc, mc * 128:(mc + 1) * 128],
                       sp[:, kc, :], start=(kc == 0), stop=(kc == 7))
                sgn(sf1[:, mc, :], pf,
                    w(pfx + "f1_s")[:, mc:mc + 1], w(pfx + "f1_c")[:, mc:mc + 1])
            sf2 = sb.tile([128, 2, BL], DT_BF, tag=pfx + "sf2")
            for mc in range(2):
                pf = ps.tile([128, BL], DT_F32, tag="p512")
                for kc in range(4):
                    mm(pf, w(pfx + "_f2_w")[:, kc, mc * 128:(mc + 1) * 128],
                       sf1[:, kc, :], start=(kc == 0), stop=(kc == 3))
                sgn(sf2[:, mc, :], pf,
                    w(pfx + "f2_s")[:, mc:mc + 1], w(pfx + "f2_c")[:, mc:mc + 1])
            jdim = 3 if pfx == "stn" else 64
            out = pp.tile([jdim, BL, jdim], DT_F32, tag=f3ctag)  # [i, b, j]
            for jc in range(jdim):
                pf = ps.tile([jdim, BL], DT_F32, tag="p512")
                for kc in range(2):
                    mm(pf, w(pfx + "_f3_w")[:, kc, jc * jdim:(jc + 1) * jdim],
                       sf2[:, kc, :], start=(kc == 0), stop=(kc == 1))
                nc.vector.tensor_scalar(out[:, :, jc], pf,
                                        w(pfx + "_f3_c")[:, jc:jc + 1], None,
                                        op0=ALU.add)
            return out

        # ================= phase A: stn on sign(x) =================
        sweep1024("stn_c1_w", "stn1", s9, a1)
        sweep1024("stn_c2_w", "stn2", a1, a2)
        c3_sweep("stn_c3_w", a2, maxsA)

        _, spA = pooled_sign(maxsA, "stn3", "spA")
        T3 = stn_mlp(spA, "stn", "T3")  # [3, b, 3] = trans[i, b, j]
        for b in range(BL):
            nc.sync.dma_start(out=tr_d[b], in_=T3[:, b, :])

        # ================= phase B: xyz transform, enc c1, fstn =================
        for t in range(NT):
            b = t // TPC
            xyzt = sb.tile([3, TPTS], DT_F32, tag="xyzt")
            nc.sync.dma_start(out=xyzt[:], in_=xyzT_d[:, col(t)])
            pXY = ps.tile([3, TPTS], DT_F32, tag="p512")
            mm(pXY, T3[:, b, :], xyzt[:])
            sgn(s9[0:3, col(t)], pXY)
        for u in range(NU):
            pE1 = psb.tile([64, 2, TPTS], DT_F32, tag="pb1024")
            mm(pE1[:, 0, :], w("enc_c1_w")[:], s9[:, col(2 * u)])
            mm(pE1[:, 1, :], w("enc_c1_w")[:], s9[:, col(2 * u + 1)])
            pE1f = pE1[:].rearrange("p h n -> p (h n)")
            sgn(sx1[:, col2(u)], pE1f, w("enc1_s")[:], w("enc1_c")[:])
            nc.scalar.activation(x1[:, col2(u)], pE1f, AF.Identity,
                                 bias=w("enc1_c")[:], scale=w("enc1_s")[:])
            nc.vector.tensor_scalar(x1[:, col2(u)], x1[:, col2(u)], 1.0, -1.0,
                                    op0=ALU.min, op1=ALU.max)
        sweep1024("fstn_c1_w", "fstn1", sx1, sfc1)
        sweep1024("fstn_c2_w", "fstn2", sfc1, sfc2)
        c3_sweep("fstn_c3_w", sfc2, maxsB)

        _, spB = pooled_sign(maxsB, "fstn3", "spB")
        T64 = stn_mlp(spB, "fstn", "T64")  # [64, b, 64] = trans_feat[i, b, j]
        for b in range(BL):
            nc.sync.dma_start(out=tf_d[b], in_=T64[:, b, :])

        # ================= phase C: feature transform, enc c2/c3, g =================
        for t in range(NT):
            b = t // TPC
            pX2 = ps.tile([64, TPTS], DT_F32, tag="p512")
            mm(pX2, T64[:, b, :], x1[:, col(t)])
            sgn(sx2[:, col(t)], pX2)
        sweep1024("enc_c2_w", "enc2", sx2, sh)
        c3_sweep("enc_c3_w", sh, maxsC)

        _, sgC = pooled_sign(maxsC, "enc3", "sgC")
        kg = sb.tile([128, 4, BL], DT_F32, tag="kg")
        cb = pp.tile([128, 4, BL], DT_F32, tag="cb")
        for mc in range(4):
            pk = ps.tile([128, BL], DT_F32, tag="p512")
            for kc in range(8):
                mm(pk, w("head_c1g_w")[:, kc, mc * 128:(mc + 1) * 128],
                   sgC[:, kc, :], start=(kc == 0), stop=(kc == 7))
            nc.scalar.copy(kg[:, mc, :], pk)
            nc.vector.tensor_scalar(cb[:, mc, :], kg[:, mc, :],
                                    w("head1_s")[:, mc:mc + 1],
                                    w("head1_c")[:, mc:mc + 1],
                                    op0=ALU.mult, op1=ALU.add)

        # ================= phase D: seg head + log_softmax =================
        # logits and exp-sums are staged so a single Ln serves the whole kernel
        zt = pp.tile([128, NT, 4, NUM_CLASS], DT_F32, tag="zt")
        ssum = pp.tile([128, NT * 4], DT_F32, tag="ssum")
        HALF = NT // 2
        for half in range(2):
            ts0 = half * HALF
            us0 = half * (NU // 2)
            sd1 = pp.tile([128, 4, HALF * TPTS], DT_BF, tag="bufx1")
            se = pp.tile([128, 2, HALF * TPTS], DT_BF, tag="buf128")
            h3 = pp.tile([128, HALF * TPTS], DT_F32, tag="buf64a")

            def hcol(t):
                return bass.ds((t - ts0) * TPTS, TPTS)

            def hcol2(u):
                return bass.ds((u - us0) * TP2, TP2)

            for mc in range(4):
                lhs = w("head_c1p_w")[:, mc * 128:(mc + 1) * 128]
                for u in range(us0, us0 + NU // 2):
                    b = u // (NU // BL)
                    pD = psb.tile([128, 2, TPTS], DT_F32, tag="pb1024")
                    mm(pD[:, 0, :], lhs, sx2[:, col(2 * u)])
                    mm(pD[:, 1, :], lhs, sx2[:, col(2 * u + 1)])
                    nc.scalar.activation(sd1[:, mc, hcol2(u)],
                                         pD[:].rearrange("p h n -> p (h n)"),
                                         AF.Sign, bias=cb[:, mc, b:b + 1],
                                         scale=w("head1_s")[:, mc:mc + 1])
            for mc in range(2):
                for t in range(ts0, ts0 + HALF):
                    pE = ps.tile([128, TPTS], DT_F32, tag="p512")
                    for kc in range(4):
                        mm(pE, w("head_c2_w")[:, kc, mc * 128:(mc + 1) * 128],
                           sd1[:, kc, hcol(t)], start=(kc == 0), stop=(kc == 3))
                    sgn(se[:, mc, hcol(t)], pE,
                        w("head2_s")[:, mc:mc + 1], w("head2_c")[:, mc:mc + 1])
            for t in range(ts0, ts0 + HALF):
                pF3 = ps.tile([128, TPTS], DT_F32, tag="p512")
                for kc in range(2):
                    mm(pF3, w("head_c3_w")[:, kc, :], se[:, kc, hcol(t)],
                       start=(kc == 0), stop=(kc == 1))
                nc.scalar.activation(h3[:, hcol(t)], pF3, AF.Identity,
                                     bias=w("head3_c")[:], scale=w("head3_s")[:])
                nc.vector.tensor_scalar(h3[:, hcol(t)], h3[:, hcol(t)], 1.0, -1.0,
                                        op0=ALU.min, op1=ALU.max)
            for t in range(ts0, ts0 + HALF):
                pL = pss.tile([128, 4, NUM_CLASS], DT_F32, tag="psmall")
                for pc in range(4):
                    mm(pL[:, pc, :],
                       h3[:, bass.ds((t - ts0) * TPTS + pc * 128, 128)],
                       w("c4_wT")[:], start=True, stop=False)
                    mm(pL[:, pc, :], ones1[:], w("c4_b")[:, pc * NUM_CLASS:
                       (pc + 1) * NUM_CLASS], start=False, stop=True)
                ex = sb.tile([128, 4, NUM_CLASS], DT_F32, tag="ex")
                nc.scalar.activation(ex[:], pL[:], AF.Exp)
                ssm = sb.tile([128, 4], DT_F32, tag="ssm")
                nc.vector.tensor_reduce(out=ssm[:], in_=ex[:], axis=AX.X, op=ALU.add)
                lsm = sb.tile([128, 4], DT_F32, tag="lsm")
                nc.scalar.activation(lsm[:], ssm[:], AF.Ln)
                oT = sb.tile([128, 4, NUM_CLASS], DT_F32, tag="oT")
                for pc in range(4):
                    nc.vector.tensor_scalar(oT[:, pc, :], pL[:, pc, :],
                                            lsm[:, pc:pc + 1], None,
                                            op0=ALU.subtract)
                nc.sync.dma_start(
                    out=lo_d[bass.ds(t * TPTS, TPTS)].rearrange(
                        "(pc p) c -> p pc c", pc=4),
                    in_=oT[:])

    nc.compile()
    return nc


# ---------------------------------------------------------------- entry point

_CACHE = {}


def _run(pos, feat, params, trace=False):
    pos = np.asarray(pos, F32)
    feat = np.asarray(feat, F32)
    wmap = _make_wmap(params)
    if "nc" not in _CACHE:
        _CACHE["nc"] = _build_program(wmap)
    nc = _CACHE["nc"]

    x = np.concatenate([pos, feat], axis=-1)               # (B, N, 9)
    sx = np.sign(x).astype(BF16)
    in_maps = []
    for i in range(M_CORES):
        sl = slice(i * BL, (i + 1) * BL)
        m = {k: v for k, v in wmap.items()}
        m["sxT"] = np.ascontiguousarray(sx[sl].transpose(2, 0, 1).reshape(9, NPTS))
        m["xyzT"] = np.ascontiguousarray(
            pos[sl].transpose(2, 0, 1).reshape(3, NPTS).astype(F32))
        in_maps.append(m)

    res = run_bass_kernel_spmd(nc, in_maps, core_ids=list(range(M_CORES)),
                               trace=trace)
    out = np.concatenate([r["logout"] for r in res.results], axis=0)
    trans = np.concatenate([r["trans_o"] for r in res.results], axis=0)
    tf = np.concatenate([r["tf_o"] for r in res.results], axis=0)
    return (out, trans, tf), res


def kernel(pos, feat, params):
    (out, trans, tf), _ = _run(pos, feat, params, trace=False)
    return out, trans, tf


# revision 16
# speedup vs baseline: 1.6336x; 1.1730x over previous
"""Bass/Trainium2 kernel for BasicBiPointNetSemSeg (binarized PointNet semantic seg).

Data-parallel over 8 NeuronCores: batch 16 -> 2 point clouds per core.
Layout on device: channels on partitions, points on the free dim.

Key transformations (all exact, not approximations):
  - sign(W) precomputed on host, stored transposed as bf16 (+-1 exact in bf16).
  - sign(x) @ sign(W).T done as bf16 matmuls (integer accumulation, exact).
  - BatchNorm + bilinear bias folded into one affine (scale s>0, bias c) applied
    by the ScalarE activation: Sign(z*s + c) produces the next layer's +-1 input
    in one op.  ht (clip) before a sign is dropped: sign(clip(x)) == sign(x).
  - Max-pool layers (stn c3 / fstn c3 / enc c3): pool the RAW integer matmul
    outputs (monotone affine with s>0 commutes with max), apply the affine once
    per cloud after pooling.  Raw values are integers |z|<=128 -> bf16 exact.
  - Global feature g enters the seg head only via sign(g) @ Wg: that term is
    constant per cloud, computed once and folded into the head-c1 bias.

Scheduling: every layer is emitted as a sweep over all 16 point tiles with the
same stationary weight, so the PE gets dense same-weight matmul bursts (keeps
the HAM clock-gate warm and lets walrus LDWEIGHTS-dedup remove reloads).
"""

import os
import sys
from contextlib import ExitStack

import numpy as np
import ml_dtypes

for _p in ("/opt/trn_rl_repo",):
    if os.path.isdir(_p) and _p not in sys.path:
        sys.path.append(_p)

import concourse.bacc as bacc
import concourse.bass as bass
import concourse.tile as tile
from concourse import mybir
from concourse.bass_utils import run_bass_kernel_spmd
import concourse.bass_utils as _bu

BF16 = ml_dtypes.bfloat16
F32 = np.float32
DT_BF = mybir.dt.bfloat16
DT_F32 = mybir.dt.float32
AF = mybir.ActivationFunctionType
ALU = mybir.AluOpType
AX = mybir.AxisListType

B, N, NUM_CLASS = 16, 4096, 13
M_CORES = 8
BL = B // M_CORES          # clouds per core
NPTS = BL * N              # points per core
TPTS = 512                 # points per tile
NT = NPTS // TPTS          # tiles per core
TPC = N // TPTS            # tiles per cloud
EPS = 1e-5

# Of the 8 reduce slots per pooled layer (pattern index mod 8), how many are
# reduced by the VectorE straight from PSUM (rest: ScalarE bf16 copy + reduce).
ND8 = int(os.environ.get("ND8", "4"))

# Let walrus drop back-to-back redundant LDWEIGHTS (sweeps reuse the weight).
if os.environ.get("LDWOPT", "0") == "1" and not getattr(_bu, "_ldw_patched", False):
    _orig_run_command = _bu.run_command

    def _run_command_ldw(argv, **kw):
        argv = ["--enable-ldw-opt=true" if a == "--enable-ldw-opt=false" else a
                for a in argv]
        return _orig_run_command(argv, **kw)

    _bu.run_command = _run_command_ldw
    _bu._ldw_patched = True


# ---------------------------------------------------------------- host prep

def _sgnT(Wdict, kc=None):
    Wt = np.ascontiguousarray(np.sign(np.asarray(Wdict["W"], F32)).T.astype(BF16))
    if kc is not None:
        Wt = np.ascontiguousarray(Wt.reshape(kc, 128, -1))
    return Wt


def _fold(lin, bn):
    g = np.asarray(bn["g"], F32)
    v = np.asarray(bn["v"], F32)
    m = np.asarray(bn["m"], F32)
    be = np.asarray(bn["be"], F32)
    b = np.asarray(lin["b"], F32)
    s = g / np.sqrt(v + EPS)
    c = (b - m) * s + be
    return s.astype(F32), c.astype(F32)


def _chunked(vec, mc):
    """[M] -> [M,1] (mc==1) or [128, mc] with [p, j] = vec[j*128+p]."""
    vec = np.asarray(vec, F32)
    if mc == 1:
        return np.ascontiguousarray(vec.reshape(-1, 1))
    return np.ascontiguousarray(vec.reshape(mc, 128).T)


def _make_wmap(params):
    p = params
    feat = p["feat"]
    stn, fstn = feat["stn"], feat["fstn"]
    w = {}

    def affine(prefix, lin, bn, mc):
        s, c = _fold(lin, bn)
        w[prefix + "_s"] = _chunked(s, mc)
        w[prefix + "_c"] = _chunked(c, mc)

    # --- stn (k=3) ---
    w["stn_c1_w"] = _sgnT(stn["c1"])                 # [9, 64]
    affine("stn1", stn["c1"], stn["b1"], 1)
    w["stn_c2_w"] = _sgnT(stn["c2"])                 # [64, 128]
    affine("stn2", stn["c2"], stn["b2"], 1)
    w["stn_c3_w"] = _sgnT(stn["c3"])                 # [128, 1024]
    affine("stn3", stn["c3"], stn["b3"], 8)
    w["stn_f1_w"] = _sgnT(stn["f1"], kc=8)           # [8,128,512]
    affine("stnf1", stn["f1"], stn["b4"], 4)
    w["stn_f2_w"] = _sgnT(stn["f2"], kc=4)           # [4,128,256]
    affine("stnf2", stn["f2"], stn["b5"], 2)
    # f3 output neurons permuted r=(i*k+j) -> r'=(j*k+i) so each M-chunk of
    # the matmul emits one transform column [i, b] directly (no reorder DMA).
    p3 = np.arange(9).reshape(3, 3).T.reshape(-1)
    w["stn_f3_w"] = np.ascontiguousarray(_sgnT(stn["f3"])[:, p3].reshape(2, 128, 9))
    c3v = (np.asarray(stn["f3"]["b"], F32) + np.eye(3, dtype=F32).reshape(-1))[p3]
    w["stn_f3_c"] = np.ascontiguousarray(c3v.reshape(3, 3).T)

    # --- fstn (k=64) ---
    w["fstn_c1_w"] = _sgnT(fstn["c1"])               # [64, 64]
    affine("fstn1", fstn["c1"], fstn["b1"], 1)
    w["fstn_c2_w"] = _sgnT(fstn["c2"])               # [64, 128]
    affine("fstn2", fstn["c2"], fstn["b2"], 1)
    w["fstn_c3_w"] = _sgnT(fstn["c3"])               # [128, 1024]
    affine("fstn3", fstn["c3"], fstn["b3"], 8)
    w["fstn_f1_w"] = _sgnT(fstn["f1"], kc=8)
    affine("fstnf1", fstn["f1"], fstn["b4"], 4)
    w["fstn_f2_w"] = _sgnT(fstn["f2"], kc=4)
    affine("fstnf2", fstn["f2"], fstn["b5"], 2)
    p64 = np.arange(4096).reshape(64, 64).T.reshape(-1)
    w["fstn_f3_w"] = np.ascontiguousarray(
        _sgnT(fstn["f3"])[:, p64].reshape(2, 128, 4096))
    c64v = (np.asarray(fstn["f3"]["b"], F32) + np.eye(64, dtype=F32).reshape(-1))[p64]
    w["fstn_f3_c"] = np.ascontiguousarray(c64v.reshape(64, 64).T)

    # --- encoder ---
    w["enc_c1_w"] = _sgnT(feat["c1"])                # [9, 64]
    affine("enc1", feat["c1"], feat["b1"], 1)
    w["enc_c2_w"] = _sgnT(feat["c2"])                # [64, 128]
    affine("enc2", feat["c2"], feat["b2"], 1)
    w["enc_c3_w"] = _sgnT(feat["c3"])                # [128, 1024]
    affine("enc3", feat["c3"], feat["b3"], 8)

    # --- seg head ---
    c1W = np.sign(np.asarray(p["c1"]["W"], F32))     # [512, 1088]
    w["head_c1g_w"] = np.ascontiguousarray(
        c1W[:, :1024].T.astype(BF16).reshape(8, 128, 512))
    w["head_c1p_w"] = np.ascontiguousarray(c1W[:, 1024:].T.astype(BF16))  # [64,512]
    affine("head1", p["c1"], p["b1"], 4)
    w["head_c2_w"] = _sgnT(p["c2"], kc=4)            # [4,128,256]
    affine("head2", p["c2"], p["b2"], 2)
    w["head_c3_w"] = _sgnT(p["c3"], kc=2)            # [2,128,128]
    affine("head3", p["c3"], p["b3"], 1)
    w["c4_wT"] = np.ascontiguousarray(np.asarray(p["c4"]["W"], F32).T)    # [128,13]
    w["c4_b"] = np.ascontiguousarray(
        np.tile(np.asarray(p["c4"]["b"], F32), 4).reshape(1, 52))
    return w


# ---------------------------------------------------------------- device program

def _build_program(wmap):
    nc = bacc.Bacc("TRN2", target_bir_lowering=False, debug=False)
    dts = {}
    for name, arr in wmap.items():
        dt = DT_BF if arr.dtype == BF16 else DT_F32
        dts[name] = nc.dram_tensor(name, list(arr.shape), dt, kind="ExternalInput").ap()
    sxT_d = nc.dram_tensor("sxT", [9, NPTS], DT_BF, kind="ExternalInput").ap()
    xyzT_d = nc.dram_tensor("xyzT", [3, NPTS], DT_F32, kind="ExternalInput").ap()
    lo_d = nc.dram_tensor("logout", [NPTS, NUM_CLASS], DT_F32, kind="ExternalOutput").ap()
    tr_d = nc.dram_tensor("trans_o", [BL, 3, 3], DT_F32, kind="ExternalOutput").ap()
    tf_d = nc.dram_tensor("tf_o", [BL, 64, 64], DT_F32, kind="ExternalOutput").ap()

    def col(t):
        return bass.ds(t * TPTS, TPTS)

    TP2 = 2 * TPTS          # 1024-point pair tiles (bf16 PSUM, one bank)
    NU = NPTS // TP2        # 8 pair tiles per core

    def col2(u):
        return bass.ds(u * TP2, TP2)

    with ExitStack() as ctx:
        tc = ctx.enter_context(tile.TileContext(nc))
        wp = ctx.enter_context(tc.tile_pool(name="wpool", bufs=1))
        pp = ctx.enter_context(tc.tile_pool(name="persist", bufs=1))
        sb = ctx.enter_context(tc.tile_pool(name="work", bufs=3))
        ps = ctx.enter_context(tc.tile_pool(name="psum", bufs=3, space="PSUM"))
        psb = ctx.enter_context(tc.tile_pool(name="psumb", bufs=2, space="PSUM"))
        pss = ctx.enter_context(tc.tile_pool(name="psumsmall", bufs=1, space="PSUM"))

        sbw = {}

        def w(name):
            if name not in sbw:
                ap = dts[name]
                arr = wmap[name]
                dt = DT_BF if arr.dtype == BF16 else DT_F32
                if arr.ndim == 3:  # [kc, 128, M] -> sbuf [128, kc, M]
                    kc = arr.shape[0]
                    t = wp.tile([128, kc, arr.shape[2]], dt, tag=name)
                    for k in range(kc):
                        nc.sync.dma_start(out=t[:, k, :], in_=ap[k])
                else:
                    t = wp.tile(list(arr.shape), dt, tag=name)
                    nc.sync.dma_start(out=t[:], in_=ap)
                sbw[name] = t
            return sbw[name]

        def mm(dst, lhsT, rhs, start=True, stop=True):
            nc.tensor.matmul(dst, lhsT, rhs, start=start, stop=stop)

        def sgn(dst, src, s=1.0, c=0.0):
            nc.scalar.activation(dst, src, AF.Sign, bias=c, scale=s)

        # persistent tensors
        s9 = pp.tile([9, NPTS], DT_BF, tag="s9")
        nc.sync.dma_start(out=s9[:], in_=sxT_d)
        # aliased buffers: same tag = same storage, disjoint lifetimes
        a1 = pp.tile([64, NPTS], DT_BF, tag="buf64a")        # phase A
        a2 = pp.tile([128, NPTS], DT_BF, tag="buf128")       # phase A
        x1 = pp.tile([64, NPTS], DT_F32, tag="bufx1")        # phase B -> C
        sx1 = pp.tile([64, NPTS], DT_BF, tag="buf64b")       # phase B
        sfc1 = pp.tile([64, NPTS], DT_BF, tag="buf64a")      # phase B (reuse a1)
        sfc2 = pp.tile([128, NPTS], DT_BF, tag="buf128")     # phase B (reuse a2)
        sx2 = pp.tile([64, NPTS], DT_BF, tag="buf64b")       # phase C -> D
        sh = pp.tile([128, NPTS], DT_BF, tag="buf128")       # phase C (reuse)
        maxsA = pp.tile([128, 8, BL, TPC], DT_F32, tag="maxsA")
        maxsB = pp.tile([128, 8, BL, TPC], DT_F32, tag="maxsB")
        maxsC = pp.tile([128, 8, BL, TPC], DT_F32, tag="maxsC")
        ones1 = pp.tile([1, 128], DT_F32, tag="ones1")
        nc.vector.memset(ones1[:], 1.0)

        def sweep1024(wname, spfx, src_all, dst_all):
            """K<=128 sign layer over 1024-point pair tiles (2-bank fp32 PSUM,
            one wide Sign per pair)."""
            lhs = w(wname)[:]
            md = lhs.shape[-1]
            s_ap = w(spfx + "_s")[:]
            c_ap = w(spfx + "_c")[:]
            for u in range(NU):
                pT = psb.tile([md, 2, TPTS], DT_F32, tag="pb1024")
                mm(pT[:, 0, :], lhs, src_all[:, col(2 * u)])
                mm(pT[:, 1, :], lhs, src_all[:, col(2 * u + 1)])
                sgn(dst_all[:, col2(u)], pT[:].rearrange("p h n -> p (h n)"),
                    s_ap, c_ap)

        def c3_sweep(wname, src_all, maxs):
            """1024-wide pooled layer: chunk-outer / pair-tile-inner, raw max."""
            for mc in range(8):
                lhs = w(wname)[:, mc * 128:(mc + 1) * 128]
                for u in range(NU):
                    b = u // (NU // BL)
                    tc0 = (2 * u) % TPC
                    pC = psb.tile([128, 2, TPTS], DT_F32, tag="pb1024")
                    mm(pC[:, 0, :], lhs, src_all[:, col(2 * u)])
                    mm(pC[:, 1, :], lhs, src_all[:, col(2 * u + 1)])
                    dst = maxs[:, mc, b, tc0:tc0 + 2]
                    if (mc * NU + u) % 8 < ND8:
                        nc.vector.tensor_reduce(out=dst, in_=pC[:], axis=AX.X,
                                                op=ALU.max)
                    else:
                        a3 = sb.tile([128, 2, TPTS], DT_BF, tag="a3")
                        nc.scalar.activation(a3[:], pC[:], AF.Copy)
                        nc.vector.tensor_reduce(out=dst, in_=a3[:], axis=AX.X,
                                                op=ALU.max)

        def pooled_sign(maxs, spfx, tag):
            pooled = pp.tile([128, 8, BL], DT_F32, tag=tag + "_raw")
            nc.vector.tensor_reduce(out=pooled[:], in_=maxs[:], axis=AX.X, op=ALU.max)
            sp = pp.tile([128, 8, BL], DT_BF, tag=tag)
            for mc in range(8):
                sgn(sp[:, mc, :], pooled[:, mc, :],
                    w(spfx + "_s")[:, mc:mc + 1], w(spfx + "_c")[:, mc:mc + 1])
            return pooled, sp

        def stn_mlp(sp, pfx, f3ctag):
            sf1 = sb.tile([128, 4, BL], DT_BF, tag=pfx + "sf1")
            for mc in range(4):
                pf = ps.tile([128, BL], DT_F32, tag="p512")
                for kc in range(8):
                    mm(pf, w(pfx + "_f1_w")[:, kc, mc * 128:(mc + 1) * 128],
                       sp[:, kc, :], start=(kc == 0), stop=(kc == 7))
                sgn(sf1[:, mc, :], pf,
                    w(pfx + "f1_s")[:, mc:mc + 1], w(pfx + "f1_c")[:, mc:mc + 1])
            sf2 = sb.tile([128, 2, BL], DT_BF, tag=pfx + "sf2")
            for mc in range(2):
                pf = ps.tile([128, BL], DT_F32, tag="p512")
                for kc in range(4):
                    mm(pf, w(pfx + "_f2_w")[:, kc, mc * 128:(mc + 1) * 128],
                       sf1[:, kc, :], start=(kc == 0), stop=(kc == 3))
                sgn(sf2[:, mc, :], pf,
                    w(pfx + "f2_s")[:, mc:mc + 1], w(pfx + "f2_c")[:, mc:mc + 1])
            jdim = 3 if pfx == "stn" else 64
            out = pp.tile([jdim, BL, jdim], DT_F32, tag=f3ctag)  # [i, b, j]
            for jc in range(jdim):
                pf = ps.tile([jdim, BL], DT_F32, tag="p512")
                for kc in range(2):
                    mm(pf, w(pfx + "_f3_w")[:, kc, jc * jdim:(jc + 1) * jdim],
                       sf2[:, kc, :], start=(kc == 0), stop=(kc == 1))
                nc.vector.tensor_scalar(out[:, :, jc], pf,
                                        w(pfx + "_f3_c")[:, jc:jc + 1], None,
                                        op0=ALU.add)
            return out

        # ================= phase A: stn on sign(x) =================
        sweep1024("stn_c1_w", "stn1", s9, a1)
        sweep1024("stn_c2_w", "stn2", a1, a2)
        c3_sweep("stn_c3_w", a2, maxsA)

        _, spA = pooled_sign(maxsA, "stn3", "spA")
        T3 = stn_mlp(spA, "stn", "T3")  # [3, b, 3] = trans[i, b, j]
        for b in range(BL):
            nc.sync.dma_start(out=tr_d[b], in_=T3[:, b, :])

        # ================= phase B: xyz transform, enc c1, fstn =================
        for t in range(NT):
            b = t // TPC
            xyzt = sb.tile([3, TPTS], DT_F32, tag="xyzt")
            nc.sync.dma_start(out=xyzt[:], in_=xyzT_d[:, col(t)])
            pXY = ps.tile([3, TPTS], DT_F32, tag="p512")
            mm(pXY, T3[:, b, :], xyzt[:])
            sgn(s9[0:3, col(t)], pXY)
        for u in range(NU):
            pE1 = psb.tile([64, 2, TPTS], DT_F32, tag="pb1024")
            mm(pE1[:, 0, :], w("enc_c1_w")[:], s9[:, col(2 * u)])
            mm(pE1[:, 1, :], w("enc_c1_w")[:], s9[:, col(2 * u + 1)])
            pE1f = pE1[:].rearrange("p h n -> p (h n)")
            sgn(sx1[:, col2(u)], pE1f, w("enc1_s")[:], w("enc1_c")[:])
            nc.scalar.activation(x1[:, col2(u)], pE1f, AF.Identity,
                                 bias=w("enc1_c")[:], scale=w("enc1_s")[:])
            nc.vector.tensor_scalar(x1[:, col2(u)], x1[:, col2(u)], 1.0, -1.0,
                                    op0=ALU.min, op1=ALU.max)
        sweep1024("fstn_c1_w", "fstn1", sx1, sfc1)
        sweep1024("fstn_c2_w", "fstn2", sfc1, sfc2)
        c3_sweep("fstn_c3_w", sfc2, maxsB)

        _, spB = pooled_sign(maxsB, "fstn3", "spB")
        T64 = stn_mlp(spB, "fstn", "T64")  # [64, b, 64] = trans_feat[i, b, j]
        for b in range(BL):
            nc.sync.dma_start(out=tf_d[b], in_=T64[:, b, :])

        # ================= phase C: feature transform, enc c2/c3, g =================
        for t in range(NT):
            b = t // TPC
            pX2 = ps.tile([64, TPTS], DT_F32, tag="p512")
            mm(pX2, T64[:, b, :], x1[:, col(t)])
            sgn(sx2[:, col(t)], pX2)
        sweep1024("enc_c2_w", "enc2", sx2, sh)
        c3_sweep("enc_c3_w", sh, maxsC)

        _, sgC = pooled_sign(maxsC, "enc3", "sgC")
        kg = sb.tile([128, 4, BL], DT_F32, tag="kg")
        cb = pp.tile([128, 4, BL], DT_F32, tag="cb")
        for mc in range(4):
            pk = ps.tile([128, BL], DT_F32, tag="p512")
            for kc in range(8):
                mm(pk, w("head_c1g_w")[:, kc, mc * 128:(mc + 1) * 128],
                   sgC[:, kc, :], start=(kc == 0), stop=(kc == 7))
            nc.scalar.copy(kg[:, mc, :], pk)
            nc.vector.tensor_scalar(cb[:, mc, :], kg[:, mc, :],
                                    w("head1_s")[:, mc:mc + 1],
                                    w("head1_c")[:, mc:mc + 1],
                                    op0=ALU.mult, op1=ALU.add)

        # ================= phase D: seg head + log_softmax =================
        # logits and exp-sums are staged so a single Ln serves the whole kernel
        zt = pp.tile([128, NT, 4, NUM_CLASS], DT_F32, tag="zt")
        ssum = pp.tile([128, NT * 4], DT_F32, tag="ssum")
        HALF = NT // 2
        for half in range(2):
            ts0 = half * HALF
            us0 = half * (NU // 2)
            sd1 = pp.tile([128, 4, HALF * TPTS], DT_BF, tag="bufx1")
            se = pp.tile([128, 2, HALF * TPTS], DT_BF, tag="buf128")
            h3 = pp.tile([128, HALF * TPTS], DT_F32, tag="buf64a")

            def hcol(t):
                return bass.ds((t - ts0) * TPTS, TPTS)

            def hcol2(u):
                return bass.ds((u - us0) * TP2, TP2)

            for mc in range(4):
                lhs = w("head_c1p_w")[:, mc * 128:(mc + 1) * 128]
                for u in range(us0, us0 + NU // 2):
                    b = u // (NU // BL)
                    pD = psb.tile([128, 2, TPTS], DT_F32, tag="pb1024")
                    mm(pD[:, 0, :], lhs, sx2[:, col(2 * u)])
                    mm(pD[:, 1, :], lhs, sx2[:, col(2 * u + 1)])
                    nc.scalar.activation(sd1[:, mc, hcol2(u)],
                                         pD[:].rearrange("p h n -> p (h n)"),
                                         AF.Sign, bias=cb[:, mc, b:b + 1],
                                         scale=w("head1_s")[:, mc:mc + 1])
            for mc in range(2):
                for t in range(ts0, ts0 + HALF):
                    pE = ps.tile([128, TPTS], DT_F32, tag="p512")
                    for kc in range(4):
                        mm(pE, w("head_c2_w")[:, kc, mc * 128:(mc + 1) * 128],
                           sd1[:, kc, hcol(t)], start=(kc == 0), stop=(kc == 3))
                    sgn(se[:, mc, hcol(t)], pE,
                        w("head2_s")[:, mc:mc + 1], w("head2_c")[:, mc:mc + 1])
            for t in range(ts0, ts0 + HALF):
                pF3 = ps.tile([128, TPTS], DT_F32, tag="p512")
                for kc in range(2):
                    mm(pF3, w("head_c3_w")[:, kc, :], se[:, kc, hcol(t)],
                       start=(kc == 0), stop=(kc == 1))
                nc.scalar.activation(h3[:, hcol(t)], pF3, AF.Identity,
                                     bias=w("head3_c")[:], scale=w("head3_s")[:])
                nc.vector.tensor_scalar(h3[:, hcol(t)], h3[:, hcol(t)], 1.0, -1.0,
                                        op0=ALU.min, op1=ALU.max)
            has_c4b = bool(np.any(wmap["c4_b"]))
            for t in range(ts0, ts0 + HALF):
                pL = pss.tile([128, 4, NUM_CLASS], DT_F32, tag="psmall")
                for pc in range(4):
                    mm(pL[:, pc, :],
                       h3[:, bass.ds((t - ts0) * TPTS + pc * 128, 128)],
                       w("c4_wT")[:], start=True, stop=not has_c4b)
                    if has_c4b:
                        mm(pL[:, pc, :], ones1[:], w("c4_b")[:, pc * NUM_CLASS:
                           (pc + 1) * NUM_CLASS], start=False, stop=True)
                nc.vector.tensor_copy(zt[:, t], pL[:])
                ex = sb.tile([128, 4, NUM_CLASS], DT_F32, tag="ex")
                nc.scalar.activation(ex[:], pL[:], AF.Exp)
                nc.vector.tensor_reduce(out=ssum[:, t * 4:(t + 1) * 4], in_=ex[:],
                                        axis=AX.X, op=ALU.add)

        lsum = pp.tile([128, NT * 4], DT_F32, tag="lsum")
        nc.scalar.activation(lsum[:], ssum[:], AF.Ln)
        for t in range(NT):
            oT = sb.tile([128, 4, NUM_CLASS], DT_F32, tag="oT")
            for pc in range(4):
                nc.vector.tensor_scalar(oT[:, pc, :], zt[:, t, pc, :],
                                        lsum[:, t * 4 + pc:t * 4 + pc + 1], None,
                                        op0=ALU.subtract)
            nc.sync.dma_start(
                out=lo_d[bass.ds(t * TPTS, TPTS)].rearrange(
                    "(pc p) c -> p pc c", pc=4),
                in_=oT[:])

    nc.compile()
    return nc


# ---------------------------------------------------------------- entry point

_CACHE = {}


def _run(pos, feat, params, trace=False):
    pos = np.asarray(pos, F32)
    feat = np.asarray(feat, F32)
    wmap = _make_wmap(params)
    if "nc" not in _CACHE:
        _CACHE["nc"] = _build_program(wmap)
    nc = _CACHE["nc"]

    x = np.concatenate([pos, feat], axis=-1)               # (B, N, 9)
    sx = np.sign(x).astype(BF16)
    in_maps = []
    for i in range(M_CORES):
        sl = slice(i * BL, (i + 1) * BL)
        m = {k: v for k, v in wmap.items()}
        m["sxT"] = np.ascontiguousarray(sx[sl].transpose(2, 0, 1).reshape(9, NPTS))
        m["xyzT"] = np.ascontiguousarray(
            pos[sl].transpose(2, 0, 1).reshape(3, NPTS).astype(F32))
        in_maps.append(m)

    res = run_bass_kernel_spmd(nc, in_maps, core_ids=list(range(M_CORES)),
                               trace=trace)
    out = np.concatenate([r["logout"] for r in res.results], axis=0)
    trans = np.concatenate([r["trans_o"] for r in res.results], axis=0)
    tf = np.concatenate([r["tf_o"] for r in res.results], axis=0)
    return (out, trans, tf), res


def kernel(pos, feat, params):
    (out, trans, tf), _ = _run(pos, feat, params, trace=False)
    return out, trans, tf


# revision 17
# speedup vs baseline: 1.6922x; 1.0359x over previous
"""Bass/Trainium2 kernel for BasicBiPointNetSemSeg (binarized PointNet semantic seg).

Data-parallel over 8 NeuronCores: batch 16 -> 2 point clouds per core.
Layout on device: channels on partitions, points on the free dim.

Key transformations (all exact, not approximations):
  - sign(W) precomputed on host, stored transposed as bf16 (+-1 exact in bf16).
  - sign(x) @ sign(W).T done as bf16 matmuls (integer accumulation, exact).
  - BatchNorm + bilinear bias folded into one affine (scale s>0, bias c) applied
    by the ScalarE activation: Sign(z*s + c) produces the next layer's +-1 input
    in one op.  ht (clip) before a sign is dropped: sign(clip(x)) == sign(x).
  - Max-pool layers (stn c3 / fstn c3 / enc c3): pool the RAW integer matmul
    outputs (monotone affine with s>0 commutes with max), apply the affine once
    per cloud after pooling.  Raw values are integers |z|<=128 -> bf16 exact.
  - Global feature g enters the seg head only via sign(g) @ Wg: that term is
    constant per cloud, computed once and folded into the head-c1 bias.

Scheduling: every layer is emitted as a sweep over all 16 point tiles with the
same stationary weight, so the PE gets dense same-weight matmul bursts (keeps
the HAM clock-gate warm and lets walrus LDWEIGHTS-dedup remove reloads).
"""

import os
import sys
from contextlib import ExitStack

import numpy as np
import ml_dtypes

for _p in ("/opt/trn_rl_repo",):
    if os.path.isdir(_p) and _p not in sys.path:
        sys.path.append(_p)

import concourse.bacc as bacc
import concourse.bass as bass
import concourse.tile as tile
from concourse import mybir
from concourse.bass_utils import run_bass_kernel_spmd
import concourse.bass_utils as _bu

BF16 = ml_dtypes.bfloat16
F32 = np.float32
DT_BF = mybir.dt.bfloat16
DT_F32 = mybir.dt.float32
AF = mybir.ActivationFunctionType
ALU = mybir.AluOpType
AX = mybir.AxisListType

B, N, NUM_CLASS = 16, 4096, 13
M_CORES = 8
BL = B // M_CORES          # clouds per core
NPTS = BL * N              # points per core
TPTS = 512                 # points per tile
NT = NPTS // TPTS          # tiles per core
TPC = N // TPTS            # tiles per cloud
EPS = 1e-5

# Of the 8 reduce slots per pooled layer (pattern index mod 8), how many are
# reduced by the VectorE straight from PSUM (rest: ScalarE bf16 copy + reduce).
ND8 = int(os.environ.get("ND8", "8"))

# Let walrus drop back-to-back redundant LDWEIGHTS (sweeps reuse the weight).
if os.environ.get("LDWOPT", "0") == "1" and not getattr(_bu, "_ldw_patched", False):
    _orig_run_command = _bu.run_command

    def _run_command_ldw(argv, **kw):
        argv = ["--enable-ldw-opt=true" if a == "--enable-ldw-opt=false" else a
                for a in argv]
        return _orig_run_command(argv, **kw)

    _bu.run_command = _run_command_ldw
    _bu._ldw_patched = True


# ---------------------------------------------------------------- host prep

def _sgnT(Wdict, kc=None):
    Wt = np.ascontiguousarray(np.sign(np.asarray(Wdict["W"], F32)).T.astype(BF16))
    if kc is not None:
        Wt = np.ascontiguousarray(Wt.reshape(kc, 128, -1))
    return Wt


def _fold(lin, bn):
    g = np.asarray(bn["g"], F32)
    v = np.asarray(bn["v"], F32)
    m = np.asarray(bn["m"], F32)
    be = np.asarray(bn["be"], F32)
    b = np.asarray(lin["b"], F32)
    s = g / np.sqrt(v + EPS)
    c = (b - m) * s + be
    return s.astype(F32), c.astype(F32)


def _chunked(vec, mc):
    """[M] -> [M,1] (mc==1) or [128, mc] with [p, j] = vec[j*128+p]."""
    vec = np.asarray(vec, F32)
    if mc == 1:
        return np.ascontiguousarray(vec.reshape(-1, 1))
    return np.ascontiguousarray(vec.reshape(mc, 128).T)


def _make_wmap(params):
    p = params
    feat = p["feat"]
    stn, fstn = feat["stn"], feat["fstn"]
    w = {}

    def affine(prefix, lin, bn, mc):
        s, c = _fold(lin, bn)
        w[prefix + "_s"] = _chunked(s, mc)
        w[prefix + "_c"] = _chunked(c, mc)

    # --- stn (k=3) ---
    w["stn_c1_w"] = _sgnT(stn["c1"])                 # [9, 64]
    affine("stn1", stn["c1"], stn["b1"], 1)
    w["stn_c2_w"] = _sgnT(stn["c2"])                 # [64, 128]
    affine("stn2", stn["c2"], stn["b2"], 1)
    w["stn_c3_w"] = _sgnT(stn["c3"])                 # [128, 1024]
    affine("stn3", stn["c3"], stn["b3"], 8)
    w["stn_f1_w"] = _sgnT(stn["f1"], kc=8)           # [8,128,512]
    affine("stnf1", stn["f1"], stn["b4"], 4)
    w["stn_f2_w"] = _sgnT(stn["f2"], kc=4)           # [4,128,256]
    affine("stnf2", stn["f2"], stn["b5"], 2)
    # f3 output neurons permuted r=(i*k+j) -> r'=(j*k+i) so each M-chunk of
    # the matmul emits one transform column [i, b] directly (no reorder DMA).
    p3 = np.arange(9).reshape(3, 3).T.reshape(-1)
    w["stn_f3_w"] = np.ascontiguousarray(_sgnT(stn["f3"])[:, p3].reshape(2, 128, 9))
    c3v = (np.asarray(stn["f3"]["b"], F32) + np.eye(3, dtype=F32).reshape(-1))[p3]
    w["stn_f3_c"] = np.ascontiguousarray(c3v.reshape(3, 3).T)

    # --- fstn (k=64) ---
    w["fstn_c1_w"] = _sgnT(fstn["c1"])               # [64, 64]
    affine("fstn1", fstn["c1"], fstn["b1"], 1)
    w["fstn_c2_w"] = _sgnT(fstn["c2"])               # [64, 128]
    affine("fstn2", fstn["c2"], fstn["b2"], 1)
    w["fstn_c3_w"] = _sgnT(fstn["c3"])               # [128, 1024]
    affine("fstn3", fstn["c3"], fstn["b3"], 8)
    w["fstn_f1_w"] = _sgnT(fstn["f1"], kc=8)
    affine("fstnf1", fstn["f1"], fstn["b4"], 4)
    w["fstn_f2_w"] = _sgnT(fstn["f2"], kc=4)
    affine("fstnf2", fstn["f2"], fstn["b5"], 2)
    p64 = np.arange(4096).reshape(64, 64).T.reshape(-1)
    w["fstn_f3_w"] = np.ascontiguousarray(
        _sgnT(fstn["f3"])[:, p64].reshape(2, 128, 4096))
    c64v = (np.asarray(fstn["f3"]["b"], F32) + np.eye(64, dtype=F32).reshape(-1))[p64]
    w["fstn_f3_c"] = np.ascontiguousarray(c64v.reshape(64, 64).T)

    # --- encoder ---
    w["enc_c1_w"] = _sgnT(feat["c1"])                # [9, 64]
    affine("enc1", feat["c1"], feat["b1"], 1)
    w["enc_c2_w"] = _sgnT(feat["c2"])                # [64, 128]
    affine("enc2", feat["c2"], feat["b2"], 1)
    w["enc_c3_w"] = _sgnT(feat["c3"])                # [128, 1024]
    affine("enc3", feat["c3"], feat["b3"], 8)

    # --- seg head ---
    c1W = np.sign(np.asarray(p["c1"]["W"], F32))     # [512, 1088]
    w["head_c1g_w"] = np.ascontiguousarray(
        c1W[:, :1024].T.astype(BF16).reshape(8, 128, 512))
    w["head_c1p_w"] = np.ascontiguousarray(c1W[:, 1024:].T.astype(BF16))  # [64,512]
    affine("head1", p["c1"], p["b1"], 4)
    w["head_c2_w"] = _sgnT(p["c2"], kc=4)            # [4,128,256]
    affine("head2", p["c2"], p["b2"], 2)
    w["head_c3_w"] = _sgnT(p["c3"], kc=2)            # [2,128,128]
    affine("head3", p["c3"], p["b3"], 1)
    w["c4_wT"] = np.ascontiguousarray(np.asarray(p["c4"]["W"], F32).T)    # [128,13]
    w["c4_b"] = np.ascontiguousarray(
        np.tile(np.asarray(p["c4"]["b"], F32), 4).reshape(1, 52))
    return w


# ---------------------------------------------------------------- device program

def _build_program(wmap):
    nc = bacc.Bacc("TRN2", target_bir_lowering=False, debug=False)
    dts = {}
    for name, arr in wmap.items():
        dt = DT_BF if arr.dtype == BF16 else DT_F32
        dts[name] = nc.dram_tensor(name, list(arr.shape), dt, kind="ExternalInput").ap()
    sxT_d = nc.dram_tensor("sxT", [9, NPTS], DT_BF, kind="ExternalInput").ap()
    xyzT_d = nc.dram_tensor("xyzT", [3, NPTS], DT_F32, kind="ExternalInput").ap()
    lo_d = nc.dram_tensor("logout", [NPTS, NUM_CLASS], DT_F32, kind="ExternalOutput").ap()
    tr_d = nc.dram_tensor("trans_o", [BL, 3, 3], DT_F32, kind="ExternalOutput").ap()
    tf_d = nc.dram_tensor("tf_o", [BL, 64, 64], DT_F32, kind="ExternalOutput").ap()

    def col(t):
        return bass.ds(t * TPTS, TPTS)

    TP2 = 2 * TPTS          # 1024-point pair tiles (bf16 PSUM, one bank)
    NU = NPTS // TP2        # 8 pair tiles per core

    def col2(u):
        return bass.ds(u * TP2, TP2)

    with ExitStack() as ctx:
        tc = ctx.enter_context(tile.TileContext(nc))
        wp = ctx.enter_context(tc.tile_pool(name="wpool", bufs=1))
        pp = ctx.enter_context(tc.tile_pool(name="persist", bufs=1))
        sb = ctx.enter_context(tc.tile_pool(name="work", bufs=3))
        ps = ctx.enter_context(tc.tile_pool(name="psum", bufs=2, space="PSUM"))
        psb = ctx.enter_context(tc.tile_pool(name="psumb", bufs=3, space="PSUM"))

        sbw = {}

        def w(name):
            if name not in sbw:
                ap = dts[name]
                arr = wmap[name]
                dt = DT_BF if arr.dtype == BF16 else DT_F32
                if arr.ndim == 3:  # [kc, 128, M] -> sbuf [128, kc, M]
                    kc = arr.shape[0]
                    t = wp.tile([128, kc, arr.shape[2]], dt, tag=name)
                    for k in range(kc):
                        nc.sync.dma_start(out=t[:, k, :], in_=ap[k])
                else:
                    t = wp.tile(list(arr.shape), dt, tag=name)
                    nc.sync.dma_start(out=t[:], in_=ap)
                sbw[name] = t
            return sbw[name]

        def mm(dst, lhsT, rhs, start=True, stop=True):
            nc.tensor.matmul(dst, lhsT, rhs, start=start, stop=stop)

        def sgn(dst, src, s=1.0, c=0.0):
            nc.scalar.activation(dst, src, AF.Sign, bias=c, scale=s)

        # persistent tensors
        s9 = pp.tile([9, NPTS], DT_BF, tag="s9")
        nc.sync.dma_start(out=s9[:], in_=sxT_d)
        # aliased buffers: same tag = same storage, disjoint lifetimes
        a1 = pp.tile([64, NPTS], DT_BF, tag="buf64a")        # phase A
        a2 = pp.tile([128, NPTS], DT_BF, tag="buf128")       # phase A
        x1 = pp.tile([64, NPTS], DT_F32, tag="bufx1")        # phase B -> C
        sx1 = pp.tile([64, NPTS], DT_BF, tag="buf64b")       # phase B
        sfc1 = pp.tile([64, NPTS], DT_BF, tag="buf64a")      # phase B (reuse a1)
        sfc2 = pp.tile([128, NPTS], DT_BF, tag="buf128")     # phase B (reuse a2)
        sx2 = pp.tile([64, NPTS], DT_BF, tag="buf64b")       # phase C -> D
        sh = pp.tile([128, NPTS], DT_BF, tag="buf128")       # phase C (reuse)
        maxsA = pp.tile([128, 8, BL, TPC], DT_F32, tag="maxsA")
        maxsB = pp.tile([128, 8, BL, TPC], DT_F32, tag="maxsB")
        maxsC = pp.tile([128, 8, BL, TPC], DT_F32, tag="maxsC")
        ones1 = pp.tile([1, 128], DT_F32, tag="ones1")
        nc.vector.memset(ones1[:], 1.0)

        def sweep1024(wname, spfx, src_all, dst_all):
            """K<=128 sign layer over 1024-point pair tiles (2-bank fp32 PSUM,
            one wide Sign per pair)."""
            lhs = w(wname)[:]
            md = lhs.shape[-1]
            s_ap = w(spfx + "_s")[:]
            c_ap = w(spfx + "_c")[:]
            for u in range(NU):
                pT = psb.tile([md, 2, TPTS], DT_F32, tag="pb1024")
                mm(pT[:, 0, :], lhs, src_all[:, col(2 * u)])
                mm(pT[:, 1, :], lhs, src_all[:, col(2 * u + 1)])
                sgn(dst_all[:, col2(u)], pT[:].rearrange("p h n -> p (h n)"),
                    s_ap, c_ap)

        def c3_sweep(wname, src_all, maxs):
            """1024-wide pooled layer: chunk-outer / pair-tile-inner, raw max."""
            for mc in range(8):
                lhs = w(wname)[:, mc * 128:(mc + 1) * 128]
                for u in range(NU):
                    b = u // (NU // BL)
                    tc0 = (2 * u) % TPC
                    pC = psb.tile([128, 2, TPTS], DT_F32, tag="pb1024")
                    mm(pC[:, 0, :], lhs, src_all[:, col(2 * u)])
                    mm(pC[:, 1, :], lhs, src_all[:, col(2 * u + 1)])
                    dst = maxs[:, mc, b, tc0:tc0 + 2]
                    if (mc * NU + u) % 8 < ND8:
                        nc.vector.tensor_reduce(out=dst, in_=pC[:], axis=AX.X,
                                                op=ALU.max)
                    else:
                        a3 = sb.tile([128, 2, TPTS], DT_BF, tag="a3")
                        nc.scalar.activation(a3[:], pC[:], AF.Copy)
                        nc.vector.tensor_reduce(out=dst, in_=a3[:], axis=AX.X,
                                                op=ALU.max)

        def pooled_sign(maxs, spfx, tag):
            pooled = pp.tile([128, 8, BL], DT_F32, tag=tag + "_raw")
            nc.vector.tensor_reduce(out=pooled[:], in_=maxs[:], axis=AX.X, op=ALU.max)
            sp = pp.tile([128, 8, BL], DT_BF, tag=tag)
            for mc in range(8):
                sgn(sp[:, mc, :], pooled[:, mc, :],
                    w(spfx + "_s")[:, mc:mc + 1], w(spfx + "_c")[:, mc:mc + 1])
            return pooled, sp

        def stn_mlp(sp, pfx, f3ctag):
            sf1 = sb.tile([128, 4, BL], DT_BF, tag=pfx + "sf1")
            for mc in range(4):
                pf = ps.tile([128, BL], DT_F32, tag="p512")
                for kc in range(8):
                    mm(pf, w(pfx + "_f1_w")[:, kc, mc * 128:(mc + 1) * 128],
                       sp[:, kc, :], start=(kc == 0), stop=(kc == 7))
                sgn(sf1[:, mc, :], pf,
                    w(pfx + "f1_s")[:, mc:mc + 1], w(pfx + "f1_c")[:, mc:mc + 1])
            sf2 = sb.tile([128, 2, BL], DT_BF, tag=pfx + "sf2")
            for mc in range(2):
                pf = ps.tile([128, BL], DT_F32, tag="p512")
                for kc in range(4):
                    mm(pf, w(pfx + "_f2_w")[:, kc, mc * 128:(mc + 1) * 128],
                       sf1[:, kc, :], start=(kc == 0), stop=(kc == 3))
                sgn(sf2[:, mc, :], pf,
                    w(pfx + "f2_s")[:, mc:mc + 1], w(pfx + "f2_c")[:, mc:mc + 1])
            jdim = 3 if pfx == "stn" else 64
            out = pp.tile([jdim, BL, jdim], DT_F32, tag=f3ctag)  # [i, b, j]
            for jc in range(jdim):
                pf = ps.tile([jdim, BL], DT_F32, tag="p512")
                for kc in range(2):
                    mm(pf, w(pfx + "_f3_w")[:, kc, jc * jdim:(jc + 1) * jdim],
                       sf2[:, kc, :], start=(kc == 0), stop=(kc == 1))
                nc.vector.tensor_scalar(out[:, :, jc], pf,
                                        w(pfx + "_f3_c")[:, jc:jc + 1], None,
                                        op0=ALU.add)
            return out

        # ================= phase A: stn on sign(x) =================
        sweep1024("stn_c1_w", "stn1", s9, a1)
        sweep1024("stn_c2_w", "stn2", a1, a2)
        c3_sweep("stn_c3_w", a2, maxsA)

        _, spA = pooled_sign(maxsA, "stn3", "spA")
        T3 = stn_mlp(spA, "stn", "T3")  # [3, b, 3] = trans[i, b, j]
        for b in range(BL):
            nc.sync.dma_start(out=tr_d[b], in_=T3[:, b, :])

        # ================= phase B: xyz transform, enc c1, fstn =================
        for t in range(NT):
            b = t // TPC
            xyzt = sb.tile([3, TPTS], DT_F32, tag="xyzt")
            nc.sync.dma_start(out=xyzt[:], in_=xyzT_d[:, col(t)])
            pXY = ps.tile([3, TPTS], DT_F32, tag="p512")
            mm(pXY, T3[:, b, :], xyzt[:])
            sgn(s9[0:3, col(t)], pXY)
        for u in range(NU):
            pE1 = psb.tile([64, 2, TPTS], DT_F32, tag="pb1024")
            mm(pE1[:, 0, :], w("enc_c1_w")[:], s9[:, col(2 * u)])
            mm(pE1[:, 1, :], w("enc_c1_w")[:], s9[:, col(2 * u + 1)])
            pE1f = pE1[:].rearrange("p h n -> p (h n)")
            sgn(sx1[:, col2(u)], pE1f, w("enc1_s")[:], w("enc1_c")[:])
            nc.scalar.activation(x1[:, col2(u)], pE1f, AF.Identity,
                                 bias=w("enc1_c")[:], scale=w("enc1_s")[:])
            nc.vector.tensor_scalar(x1[:, col2(u)], x1[:, col2(u)], 1.0, -1.0,
                                    op0=ALU.min, op1=ALU.max)
        sweep1024("fstn_c1_w", "fstn1", sx1, sfc1)
        sweep1024("fstn_c2_w", "fstn2", sfc1, sfc2)
        c3_sweep("fstn_c3_w", sfc2, maxsB)

        _, spB = pooled_sign(maxsB, "fstn3", "spB")
        T64 = stn_mlp(spB, "fstn", "T64")  # [64, b, 64] = trans_feat[i, b, j]
        for b in range(BL):
            nc.sync.dma_start(out=tf_d[b], in_=T64[:, b, :])

        # ================= phase C: feature transform, enc c2/c3, g =================
        for t in range(NT):
            b = t // TPC
            pX2 = ps.tile([64, TPTS], DT_F32, tag="p512")
            mm(pX2, T64[:, b, :], x1[:, col(t)])
            sgn(sx2[:, col(t)], pX2)
        sweep1024("enc_c2_w", "enc2", sx2, sh)
        c3_sweep("enc_c3_w", sh, maxsC)

        _, sgC = pooled_sign(maxsC, "enc3", "sgC")
        kg = sb.tile([128, 4, BL], DT_F32, tag="kg")
        cb = pp.tile([128, 4, BL], DT_F32, tag="cb")
        for mc in range(4):
            pk = ps.tile([128, BL], DT_F32, tag="p512")
            for kc in range(8):
                mm(pk, w("head_c1g_w")[:, kc, mc * 128:(mc + 1) * 128],
                   sgC[:, kc, :], start=(kc == 0), stop=(kc == 7))
            nc.scalar.copy(kg[:, mc, :], pk)
            nc.vector.tensor_scalar(cb[:, mc, :], kg[:, mc, :],
                                    w("head1_s")[:, mc:mc + 1],
                                    w("head1_c")[:, mc:mc + 1],
                                    op0=ALU.mult, op1=ALU.add)

        # ================= phase D: seg head + log_softmax =================
        # logits and exp-sums are staged so a single Ln serves the whole kernel
        zt = pp.tile([128, NT, 4, NUM_CLASS], DT_F32, tag="zt")
        ssum = pp.tile([128, NT * 4], DT_F32, tag="ssum")
        HALF = NT // 2
        for half in range(2):
            ts0 = half * HALF
            us0 = half * (NU // 2)
            sd1 = pp.tile([128, 4, HALF * TPTS], DT_BF, tag="bufx1")
            se = pp.tile([128, 2, HALF * TPTS], DT_BF, tag="buf128")
            h3 = pp.tile([128, HALF * TPTS], DT_F32, tag="buf64a")

            def hcol(t):
                return bass.ds((t - ts0) * TPTS, TPTS)

            def hcol2(u):
                return bass.ds((u - us0) * TP2, TP2)

            for mc in range(4):
                lhs = w("head_c1p_w")[:, mc * 128:(mc + 1) * 128]
                for u in range(us0, us0 + NU // 2):
                    b = u // (NU // BL)
                    pD = psb.tile([128, 2, TPTS], DT_F32, tag="pb1024")
                    mm(pD[:, 0, :], lhs, sx2[:, col(2 * u)])
                    mm(pD[:, 1, :], lhs, sx2[:, col(2 * u + 1)])
                    nc.scalar.activation(sd1[:, mc, hcol2(u)],
                                         pD[:].rearrange("p h n -> p (h n)"),
                                         AF.Sign, bias=cb[:, mc, b:b + 1],
                                         scale=w("head1_s")[:, mc:mc + 1])
            for mc in range(2):
                for t in range(ts0, ts0 + HALF):
                    pE = ps.tile([128, TPTS], DT_F32, tag="p512")
                    for kc in range(4):
                        mm(pE, w("head_c2_w")[:, kc, mc * 128:(mc + 1) * 128],
                           sd1[:, kc, hcol(t)], start=(kc == 0), stop=(kc == 3))
                    sgn(se[:, mc, hcol(t)], pE,
                        w("head2_s")[:, mc:mc + 1], w("head2_c")[:, mc:mc + 1])
            for t in range(ts0, ts0 + HALF):
                pF3 = ps.tile([128, TPTS], DT_F32, tag="p512")
                for kc in range(2):
                    mm(pF3, w("head_c3_w")[:, kc, :], se[:, kc, hcol(t)],
                       start=(kc == 0), stop=(kc == 1))
                nc.scalar.activation(h3[:, hcol(t)], pF3, AF.Identity,
                                     bias=w("head3_c")[:], scale=w("head3_s")[:])
                nc.vector.tensor_scalar(h3[:, hcol(t)], h3[:, hcol(t)], 1.0, -1.0,
                                        op0=ALU.min, op1=ALU.max)
            has_c4b = bool(np.any(wmap["c4_b"]))
            for t in range(ts0, ts0 + HALF):
                pL = ps.tile([128, 4, NUM_CLASS], DT_F32, tag="p512")
                for pc in range(4):
                    mm(pL[:, pc, :],
                       h3[:, bass.ds((t - ts0) * TPTS + pc * 128, 128)],
                       w("c4_wT")[:], start=True, stop=not has_c4b)
                    if has_c4b:
                        mm(pL[:, pc, :], ones1[:], w("c4_b")[:, pc * NUM_CLASS:
                           (pc + 1) * NUM_CLASS], start=False, stop=True)
                nc.vector.tensor_copy(zt[:, t], pL[:])
                ex = sb.tile([128, 4, NUM_CLASS], DT_F32, tag="ex")
                nc.scalar.activation(ex[:], pL[:], AF.Exp)
                nc.vector.tensor_reduce(out=ssum[:, t * 4:(t + 1) * 4], in_=ex[:],
                                        axis=AX.X, op=ALU.add)
            lsum = sb.tile([128, HALF * 4], DT_F32, tag="lsum")
            nc.scalar.activation(lsum[:], ssum[:, ts0 * 4:(ts0 + HALF) * 4], AF.Ln)
            for t in range(ts0, ts0 + HALF):
                oT = sb.tile([128, 4, NUM_CLASS], DT_F32, tag="oT")
                for pc in range(4):
                    tl = (t - ts0) * 4 + pc
                    nc.vector.tensor_scalar(oT[:, pc, :], zt[:, t, pc, :],
                                            lsum[:, tl:tl + 1], None,
                                            op0=ALU.subtract)
                nc.sync.dma_start(
                    out=lo_d[bass.ds(t * TPTS, TPTS)].rearrange(
                        "(pc p) c -> p pc c", pc=4),
                    in_=oT[:])

    nc.compile()
    return nc


# ---------------------------------------------------------------- entry point

_CACHE = {}


def _run(pos, feat, params, trace=False):
    pos = np.asarray(pos, F32)
    feat = np.asarray(feat, F32)
    wmap = _make_wmap(params)
    if "nc" not in _CACHE:
        _CACHE["nc"] = _build_program(wmap)
    nc = _CACHE["nc"]

    x = np.concatenate([pos, feat], axis=-1)               # (B, N, 9)
    sx = np.sign(x).astype(BF16)
    in_maps = []
    for i in range(M_CORES):
        sl = slice(i * BL, (i + 1) * BL)
        m = {k: v for k, v in wmap.items()}
        m["sxT"] = np.ascontiguousarray(sx[sl].transpose(2, 0, 1).reshape(9, NPTS))
        m["xyzT"] = np.ascontiguousarray(
            pos[sl].transpose(2, 0, 1).reshape(3, NPTS).astype(F32))
        in_maps.append(m)

    res = run_bass_kernel_spmd(nc, in_maps, core_ids=list(range(M_CORES)),
                               trace=trace)
    out = np.concatenate([r["logout"] for r in res.results], axis=0)
    trans = np.concatenate([r["trans_o"] for r in res.results], axis=0)
    tf = np.concatenate([r["tf_o"] for r in res.results], axis=0)
    return (out, trans, tf), res


def kernel(pos, feat, params):
    (out, trans, tf), _ = _run(pos, feat, params, trace=False)
    return out, trans, tf


# revision 18
# speedup vs baseline: 1.7525x; 1.0356x over previous
"""Bass/Trainium2 kernel for BasicBiPointNetSemSeg (binarized PointNet semantic seg).

Data-parallel over 8 NeuronCores: batch 16 -> 2 point clouds per core.
Layout on device: channels on partitions, points on the free dim.

Key transformations (all exact, not approximations):
  - sign(W) precomputed on host, stored transposed as bf16 (+-1 exact in bf16).
  - sign(x) @ sign(W).T done as bf16 matmuls (integer accumulation, exact).
  - BatchNorm + bilinear bias folded into one affine (scale s>0, bias c) applied
    by the ScalarE activation: Sign(z*s + c) produces the next layer's +-1 input
    in one op.  ht (clip) before a sign is dropped: sign(clip(x)) == sign(x).
  - Max-pool layers (stn c3 / fstn c3 / enc c3): pool the RAW integer matmul
    outputs (monotone affine with s>0 commutes with max), apply the affine once
    per cloud after pooling.  Raw values are integers |z|<=128 -> bf16 exact.
  - Global feature g enters the seg head only via sign(g) @ Wg: that term is
    constant per cloud, computed once and folded into the head-c1 bias.

Scheduling: every layer is emitted as a sweep over all 16 point tiles with the
same stationary weight, so the PE gets dense same-weight matmul bursts (keeps
the HAM clock-gate warm and lets walrus LDWEIGHTS-dedup remove reloads).
"""

import os
import sys
from contextlib import ExitStack

import numpy as np
import ml_dtypes

for _p in ("/opt/trn_rl_repo",):
    if os.path.isdir(_p) and _p not in sys.path:
        sys.path.append(_p)

import concourse.bacc as bacc
import concourse.bass as bass
import concourse.tile as tile
from concourse import mybir
from concourse.bass_utils import run_bass_kernel_spmd
import concourse.bass_utils as _bu

BF16 = ml_dtypes.bfloat16
F32 = np.float32
DT_BF = mybir.dt.bfloat16
DT_F32 = mybir.dt.float32
AF = mybir.ActivationFunctionType
ALU = mybir.AluOpType
AX = mybir.AxisListType

B, N, NUM_CLASS = 16, 4096, 13
M_CORES = 8
BL = B // M_CORES          # clouds per core
NPTS = BL * N              # points per core
TPTS = 512                 # points per tile
NT = NPTS // TPTS          # tiles per core
TPC = N // TPTS            # tiles per cloud
EPS = 1e-5

# Of the 8 reduce slots per pooled layer (pattern index mod 8), how many are
# reduced by the VectorE straight from PSUM (rest: ScalarE bf16 copy + reduce).
ND8 = int(os.environ.get("ND8", "6"))

# Let walrus drop back-to-back redundant LDWEIGHTS (sweeps reuse the weight).
if os.environ.get("LDWOPT", "0") == "1" and not getattr(_bu, "_ldw_patched", False):
    _orig_run_command = _bu.run_command

    def _run_command_ldw(argv, **kw):
        argv = ["--enable-ldw-opt=true" if a == "--enable-ldw-opt=false" else a
                for a in argv]
        return _orig_run_command(argv, **kw)

    _bu.run_command = _run_command_ldw
    _bu._ldw_patched = True


# ---------------------------------------------------------------- host prep

def _sgnT(Wdict, kc=None):
    Wt = np.ascontiguousarray(np.sign(np.asarray(Wdict["W"], F32)).T.astype(BF16))
    if kc is not None:
        Wt = np.ascontiguousarray(Wt.reshape(kc, 128, -1))
    return Wt


def _fold(lin, bn):
    g = np.asarray(bn["g"], F32)
    v = np.asarray(bn["v"], F32)
    m = np.asarray(bn["m"], F32)
    be = np.asarray(bn["be"], F32)
    b = np.asarray(lin["b"], F32)
    s = g / np.sqrt(v + EPS)
    c = (b - m) * s + be
    return s.astype(F32), c.astype(F32)


def _chunked(vec, mc):
    """[M] -> [M,1] (mc==1) or [128, mc] with [p, j] = vec[j*128+p]."""
    vec = np.asarray(vec, F32)
    if mc == 1:
        return np.ascontiguousarray(vec.reshape(-1, 1))
    return np.ascontiguousarray(vec.reshape(mc, 128).T)


def _make_wmap(params):
    p = params
    feat = p["feat"]
    stn, fstn = feat["stn"], feat["fstn"]
    w = {}

    def affine(prefix, lin, bn, mc):
        s, c = _fold(lin, bn)
        w[prefix + "_s"] = _chunked(s, mc)
        w[prefix + "_c"] = _chunked(c, mc)

    # --- stn (k=3) ---
    w["stn_c1_w"] = _sgnT(stn["c1"])                 # [9, 64]
    affine("stn1", stn["c1"], stn["b1"], 1)
    w["stn_c2_w"] = _sgnT(stn["c2"])                 # [64, 128]
    affine("stn2", stn["c2"], stn["b2"], 1)
    w["stn_c3_w"] = _sgnT(stn["c3"])                 # [128, 1024]
    affine("stn3", stn["c3"], stn["b3"], 8)
    w["stn_f1_w"] = _sgnT(stn["f1"], kc=8)           # [8,128,512]
    affine("stnf1", stn["f1"], stn["b4"], 4)
    w["stn_f2_w"] = _sgnT(stn["f2"], kc=4)           # [4,128,256]
    affine("stnf2", stn["f2"], stn["b5"], 2)
    # f3 output neurons permuted r=(i*k+j) -> r'=(j*k+i) so each M-chunk of
    # the matmul emits one transform column [i, b] directly (no reorder DMA).
    p3 = np.arange(9).reshape(3, 3).T.reshape(-1)
    w["stn_f3_w"] = np.ascontiguousarray(_sgnT(stn["f3"])[:, p3].reshape(2, 128, 9))
    c3v = (np.asarray(stn["f3"]["b"], F32) + np.eye(3, dtype=F32).reshape(-1))[p3]
    w["stn_f3_c"] = np.ascontiguousarray(c3v.reshape(3, 3).T)

    # --- fstn (k=64) ---
    w["fstn_c1_w"] = _sgnT(fstn["c1"])               # [64, 64]
    affine("fstn1", fstn["c1"], fstn["b1"], 1)
    w["fstn_c2_w"] = _sgnT(fstn["c2"])               # [64, 128]
    affine("fstn2", fstn["c2"], fstn["b2"], 1)
    w["fstn_c3_w"] = _sgnT(fstn["c3"])               # [128, 1024]
    affine("fstn3", fstn["c3"], fstn["b3"], 8)
    w["fstn_f1_w"] = _sgnT(fstn["f1"], kc=8)
    affine("fstnf1", fstn["f1"], fstn["b4"], 4)
    w["fstn_f2_w"] = _sgnT(fstn["f2"], kc=4)
    affine("fstnf2", fstn["f2"], fstn["b5"], 2)
    p64 = np.arange(4096).reshape(64, 64).T.reshape(-1)
    w["fstn_f3_w"] = np.ascontiguousarray(
        _sgnT(fstn["f3"])[:, p64].reshape(2, 128, 4096))
    c64v = (np.asarray(fstn["f3"]["b"], F32) + np.eye(64, dtype=F32).reshape(-1))[p64]
    w["fstn_f3_c"] = np.ascontiguousarray(c64v.reshape(64, 64).T)

    # --- encoder ---
    w["enc_c1_w"] = _sgnT(feat["c1"])                # [9, 64]
    affine("enc1", feat["c1"], feat["b1"], 1)
    w["enc_c2_w"] = _sgnT(feat["c2"])                # [64, 128]
    affine("enc2", feat["c2"], feat["b2"], 1)
    w["enc_c3_w"] = _sgnT(feat["c3"])                # [128, 1024]
    affine("enc3", feat["c3"], feat["b3"], 8)

    # --- seg head ---
    c1W = np.sign(np.asarray(p["c1"]["W"], F32))     # [512, 1088]
    w["head_c1g_w"] = np.ascontiguousarray(
        c1W[:, :1024].T.astype(BF16).reshape(8, 128, 512))
    w["head_c1p_w"] = np.ascontiguousarray(c1W[:, 1024:].T.astype(BF16))  # [64,512]
    affine("head1", p["c1"], p["b1"], 4)
    w["head_c2_w"] = _sgnT(p["c2"], kc=4)            # [4,128,256]
    affine("head2", p["c2"], p["b2"], 2)
    w["head_c3_w"] = _sgnT(p["c3"], kc=2)            # [2,128,128]
    affine("head3", p["c3"], p["b3"], 1)
    w["c4_wT"] = np.ascontiguousarray(np.asarray(p["c4"]["W"], F32).T.astype(BF16))
    w["c4_b"] = np.ascontiguousarray(
        np.tile(np.asarray(p["c4"]["b"], F32), 4).reshape(1, 52))
    return w


# ---------------------------------------------------------------- device program

def _build_program(wmap):
    nc = bacc.Bacc("TRN2", target_bir_lowering=False, debug=False)
    dts = {}
    for name, arr in wmap.items():
        dt = DT_BF if arr.dtype == BF16 else DT_F32
        dts[name] = nc.dram_tensor(name, list(arr.shape), dt, kind="ExternalInput").ap()
    sxT_d = nc.dram_tensor("sxT", [9, NPTS], DT_BF, kind="ExternalInput").ap()
    xyzT_d = nc.dram_tensor("xyzT", [3, NPTS], DT_F32, kind="ExternalInput").ap()
    lo_d = nc.dram_tensor("logout", [NPTS, NUM_CLASS], DT_F32, kind="ExternalOutput").ap()
    tr_d = nc.dram_tensor("trans_o", [BL, 3, 3], DT_F32, kind="ExternalOutput").ap()
    tf_d = nc.dram_tensor("tf_o", [BL, 64, 64], DT_F32, kind="ExternalOutput").ap()

    def col(t):
        return bass.ds(t * TPTS, TPTS)

    TP2 = 2 * TPTS          # 1024-point pair tiles (bf16 PSUM, one bank)
    NU = NPTS // TP2        # 8 pair tiles per core

    def col2(u):
        return bass.ds(u * TP2, TP2)

    with ExitStack() as ctx:
        tc = ctx.enter_context(tile.TileContext(nc))
        wp = ctx.enter_context(tc.tile_pool(name="wpool", bufs=1))
        pp = ctx.enter_context(tc.tile_pool(name="persist", bufs=1))
        sb = ctx.enter_context(tc.tile_pool(name="work", bufs=3))
        ps = ctx.enter_context(tc.tile_pool(name="psum", bufs=2, space="PSUM"))
        psb = ctx.enter_context(tc.tile_pool(name="psumb", bufs=3, space="PSUM"))

        sbw = {}

        def w(name):
            if name not in sbw:
                ap = dts[name]
                arr = wmap[name]
                dt = DT_BF if arr.dtype == BF16 else DT_F32
                if arr.ndim == 3:  # [kc, 128, M] -> sbuf [128, kc, M]
                    kc = arr.shape[0]
                    t = wp.tile([128, kc, arr.shape[2]], dt, tag=name)
                    for k in range(kc):
                        nc.sync.dma_start(out=t[:, k, :], in_=ap[k])
                else:
                    t = wp.tile(list(arr.shape), dt, tag=name)
                    nc.sync.dma_start(out=t[:], in_=ap)
                sbw[name] = t
            return sbw[name]

        def mm(dst, lhsT, rhs, start=True, stop=True):
            nc.tensor.matmul(dst, lhsT, rhs, start=start, stop=stop)

        def sgn(dst, src, s=1.0, c=0.0):
            nc.scalar.activation(dst, src, AF.Sign, bias=c, scale=s)

        # persistent tensors
        s9 = pp.tile([9, NPTS], DT_BF, tag="s9")
        nc.sync.dma_start(out=s9[:], in_=sxT_d)
        # aliased buffers: same tag = same storage, disjoint lifetimes
        a1 = pp.tile([64, NPTS], DT_BF, tag="buf64a")        # phase A
        a2 = pp.tile([128, NPTS], DT_BF, tag="buf128")       # phase A
        x1 = pp.tile([64, NPTS], DT_F32, tag="bufx1")        # phase B -> C
        sx1 = pp.tile([64, NPTS], DT_BF, tag="buf64b")       # phase B
        sfc1 = pp.tile([64, NPTS], DT_BF, tag="buf64a")      # phase B (reuse a1)
        sfc2 = pp.tile([128, NPTS], DT_BF, tag="buf128")     # phase B (reuse a2)
        sx2 = pp.tile([64, NPTS], DT_BF, tag="buf64b")       # phase C -> D
        sh = pp.tile([128, NPTS], DT_BF, tag="buf128")       # phase C (reuse)
        maxsA = pp.tile([128, 8, BL, TPC], DT_F32, tag="maxsA")
        maxsB = pp.tile([128, 8, BL, TPC], DT_F32, tag="maxsB")
        maxsC = pp.tile([128, 8, BL, TPC], DT_F32, tag="maxsC")
        ones1 = pp.tile([1, 128], DT_F32, tag="ones1")
        nc.vector.memset(ones1[:], 1.0)

        def sweep1024(wname, spfx, src_all, dst_all):
            """K<=128 sign layer over 1024-point pair tiles (2-bank fp32 PSUM,
            one wide Sign per pair)."""
            lhs = w(wname)[:]
            md = lhs.shape[-1]
            s_ap = w(spfx + "_s")[:]
            c_ap = w(spfx + "_c")[:]
            for u in range(NU):
                pT = psb.tile([md, 2, TPTS], DT_F32, tag="pb1024")
                mm(pT[:, 0, :], lhs, src_all[:, col(2 * u)])
                mm(pT[:, 1, :], lhs, src_all[:, col(2 * u + 1)])
                sgn(dst_all[:, col2(u)], pT[:].rearrange("p h n -> p (h n)"),
                    s_ap, c_ap)

        def c3_sweep(wname, src_all, maxs):
            """1024-wide pooled layer: chunk-outer / pair-tile-inner, raw max."""
            for mc in range(8):
                lhs = w(wname)[:, mc * 128:(mc + 1) * 128]
                for u in range(NU):
                    b = u // (NU // BL)
                    tc0 = (2 * u) % TPC
                    pC = psb.tile([128, 2, TPTS], DT_F32, tag="pb1024")
                    mm(pC[:, 0, :], lhs, src_all[:, col(2 * u)])
                    mm(pC[:, 1, :], lhs, src_all[:, col(2 * u + 1)])
                    dst = maxs[:, mc, b, tc0:tc0 + 2]
                    if (mc * NU + u) % 8 < ND8:
                        nc.vector.tensor_reduce(out=dst, in_=pC[:], axis=AX.X,
                                                op=ALU.max)
                    else:
                        a3 = sb.tile([128, 2, TPTS], DT_BF, tag="a3")
                        nc.scalar.activation(a3[:], pC[:], AF.Copy)
                        nc.vector.tensor_reduce(out=dst, in_=a3[:], axis=AX.X,
                                                op=ALU.max)

        def pooled_sign(maxs, spfx, tag):
            pooled = pp.tile([128, 8, BL], DT_F32, tag=tag + "_raw")
            nc.vector.tensor_reduce(out=pooled[:], in_=maxs[:], axis=AX.X, op=ALU.max)
            sp = pp.tile([128, 8, BL], DT_BF, tag=tag)
            for mc in range(8):
                sgn(sp[:, mc, :], pooled[:, mc, :],
                    w(spfx + "_s")[:, mc:mc + 1], w(spfx + "_c")[:, mc:mc + 1])
            return pooled, sp

        def stn_mlp(sp, pfx, f3ctag):
            sf1 = sb.tile([128, 4, BL], DT_BF, tag=pfx + "sf1")
            for mc in range(4):
                pf = ps.tile([128, BL], DT_F32, tag="p512")
                for kc in range(8):
                    mm(pf, w(pfx + "_f1_w")[:, kc, mc * 128:(mc + 1) * 128],
                       sp[:, kc, :], start=(kc == 0), stop=(kc == 7))
                sgn(sf1[:, mc, :], pf,
                    w(pfx + "f1_s")[:, mc:mc + 1], w(pfx + "f1_c")[:, mc:mc + 1])
            sf2 = sb.tile([128, 2, BL], DT_BF, tag=pfx + "sf2")
            for mc in range(2):
                pf = ps.tile([128, BL], DT_F32, tag="p512")
                for kc in range(4):
                    mm(pf, w(pfx + "_f2_w")[:, kc, mc * 128:(mc + 1) * 128],
                       sf1[:, kc, :], start=(kc == 0), stop=(kc == 3))
                sgn(sf2[:, mc, :], pf,
                    w(pfx + "f2_s")[:, mc:mc + 1], w(pfx + "f2_c")[:, mc:mc + 1])
            jdim = 3 if pfx == "stn" else 64
            out = pp.tile([jdim, BL, jdim], DT_F32, tag=f3ctag)  # [i, b, j]
            for jc in range(jdim):
                pf = ps.tile([jdim, BL], DT_F32, tag="p512")
                for kc in range(2):
                    mm(pf, w(pfx + "_f3_w")[:, kc, jc * jdim:(jc + 1) * jdim],
                       sf2[:, kc, :], start=(kc == 0), stop=(kc == 1))
                nc.vector.tensor_scalar(out[:, :, jc], pf,
                                        w(pfx + "_f3_c")[:, jc:jc + 1], None,
                                        op0=ALU.add)
            return out

        # ================= phase A: stn on sign(x) =================
        sweep1024("stn_c1_w", "stn1", s9, a1)
        sweep1024("stn_c2_w", "stn2", a1, a2)
        c3_sweep("stn_c3_w", a2, maxsA)

        _, spA = pooled_sign(maxsA, "stn3", "spA")
        T3 = stn_mlp(spA, "stn", "T3")  # [3, b, 3] = trans[i, b, j]
        for b in range(BL):
            nc.sync.dma_start(out=tr_d[b], in_=T3[:, b, :])

        # ================= phase B: xyz transform, enc c1, fstn =================
        for t in range(NT):
            b = t // TPC
            xyzt = sb.tile([3, TPTS], DT_F32, tag="xyzt")
            nc.sync.dma_start(out=xyzt[:], in_=xyzT_d[:, col(t)])
            pXY = ps.tile([3, TPTS], DT_F32, tag="p512")
            mm(pXY, T3[:, b, :], xyzt[:])
            sgn(s9[0:3, col(t)], pXY)
        for u in range(NU):
            pE1 = psb.tile([64, 2, TPTS], DT_F32, tag="pb1024")
            mm(pE1[:, 0, :], w("enc_c1_w")[:], s9[:, col(2 * u)])
            mm(pE1[:, 1, :], w("enc_c1_w")[:], s9[:, col(2 * u + 1)])
            pE1f = pE1[:].rearrange("p h n -> p (h n)")
            sgn(sx1[:, col2(u)], pE1f, w("enc1_s")[:], w("enc1_c")[:])
            nc.scalar.activation(x1[:, col2(u)], pE1f, AF.Identity,
                                 bias=w("enc1_c")[:], scale=w("enc1_s")[:])
            nc.vector.tensor_scalar(x1[:, col2(u)], x1[:, col2(u)], 1.0, -1.0,
                                    op0=ALU.min, op1=ALU.max)
        sweep1024("fstn_c1_w", "fstn1", sx1, sfc1)
        sweep1024("fstn_c2_w", "fstn2", sfc1, sfc2)
        c3_sweep("fstn_c3_w", sfc2, maxsB)

        _, spB = pooled_sign(maxsB, "fstn3", "spB")
        T64 = stn_mlp(spB, "fstn", "T64")  # [64, b, 64] = trans_feat[i, b, j]
        for b in range(BL):
            nc.sync.dma_start(out=tf_d[b], in_=T64[:, b, :])

        # ================= phase C: feature transform, enc c2/c3, g =================
        for t in range(NT):
            b = t // TPC
            pX2 = ps.tile([64, TPTS], DT_F32, tag="p512")
            mm(pX2, T64[:, b, :], x1[:, col(t)])
            sgn(sx2[:, col(t)], pX2)
        sweep1024("enc_c2_w", "enc2", sx2, sh)
        c3_sweep("enc_c3_w", sh, maxsC)

        _, sgC = pooled_sign(maxsC, "enc3", "sgC")
        kg = sb.tile([128, 4, BL], DT_F32, tag="kg")
        cb = pp.tile([128, 4, BL], DT_F32, tag="cb")
        for mc in range(4):
            pk = ps.tile([128, BL], DT_F32, tag="p512")
            for kc in range(8):
                mm(pk, w("head_c1g_w")[:, kc, mc * 128:(mc + 1) * 128],
                   sgC[:, kc, :], start=(kc == 0), stop=(kc == 7))
            nc.scalar.copy(kg[:, mc, :], pk)
            nc.vector.tensor_scalar(cb[:, mc, :], kg[:, mc, :],
                                    w("head1_s")[:, mc:mc + 1],
                                    w("head1_c")[:, mc:mc + 1],
                                    op0=ALU.mult, op1=ALU.add)

        # ================= phase D: seg head + log_softmax =================
        # logits and exp-sums are staged so a single Ln serves the whole kernel
        zt = pp.tile([128, NT, 4, NUM_CLASS], DT_F32, tag="zt")
        ssum = pp.tile([128, NT * 4], DT_F32, tag="ssum")
        HALF = NT // 2
        for half in range(2):
            ts0 = half * HALF
            us0 = half * (NU // 2)
            sd1 = pp.tile([128, 4, HALF * TPTS], DT_BF, tag="bufx1")
            se = pp.tile([128, 2, HALF * TPTS], DT_BF, tag="buf128")
            h3 = pp.tile([128, HALF * TPTS], DT_BF, tag="buf64a")

            def hcol(t):
                return bass.ds((t - ts0) * TPTS, TPTS)

            def hcol2(u):
                return bass.ds((u - us0) * TP2, TP2)

            for mc in range(4):
                lhs = w("head_c1p_w")[:, mc * 128:(mc + 1) * 128]
                for u in range(us0, us0 + NU // 2):
                    b = u // (NU // BL)
                    pD = psb.tile([128, 2, TPTS], DT_F32, tag="pb1024")
                    mm(pD[:, 0, :], lhs, sx2[:, col(2 * u)])
                    mm(pD[:, 1, :], lhs, sx2[:, col(2 * u + 1)])
                    nc.scalar.activation(sd1[:, mc, hcol2(u)],
                                         pD[:].rearrange("p h n -> p (h n)"),
                                         AF.Sign, bias=cb[:, mc, b:b + 1],
                                         scale=w("head1_s")[:, mc:mc + 1])
            for mc in range(2):
                for t in range(ts0, ts0 + HALF):
                    pE = ps.tile([128, TPTS], DT_F32, tag="p512")
                    for kc in range(4):
                        mm(pE, w("head_c2_w")[:, kc, mc * 128:(mc + 1) * 128],
                           sd1[:, kc, hcol(t)], start=(kc == 0), stop=(kc == 3))
                    sgn(se[:, mc, hcol(t)], pE,
                        w("head2_s")[:, mc:mc + 1], w("head2_c")[:, mc:mc + 1])
            for t in range(ts0, ts0 + HALF):
                pF3 = ps.tile([128, TPTS], DT_F32, tag="p512")
                for kc in range(2):
                    mm(pF3, w("head_c3_w")[:, kc, :], se[:, kc, hcol(t)],
                       start=(kc == 0), stop=(kc == 1))
                nc.scalar.activation(h3[:, hcol(t)], pF3, AF.Identity,
                                     bias=w("head3_c")[:], scale=w("head3_s")[:])
                nc.vector.tensor_scalar(h3[:, hcol(t)], h3[:, hcol(t)], 1.0, -1.0,
                                        op0=ALU.min, op1=ALU.max)
            has_c4b = bool(np.any(wmap["c4_b"]))
            for t in range(ts0, ts0 + HALF):
                pL = ps.tile([128, 4, NUM_CLASS], DT_F32, tag="p512")
                for pc in range(4):
                    mm(pL[:, pc, :],
                       h3[:, bass.ds((t - ts0) * TPTS + pc * 128, 128)],
                       w("c4_wT")[:], start=True, stop=not has_c4b)
                    if has_c4b:
                        mm(pL[:, pc, :], ones1[:], w("c4_b")[:, pc * NUM_CLASS:
                           (pc + 1) * NUM_CLASS], start=False, stop=True)
                nc.vector.tensor_copy(zt[:, t], pL[:])
                ex = sb.tile([128, 4, NUM_CLASS], DT_F32, tag="ex")
                nc.scalar.activation(ex[:], pL[:], AF.Exp)
                nc.vector.tensor_reduce(out=ssum[:, t * 4:(t + 1) * 4], in_=ex[:],
                                        axis=AX.X, op=ALU.add)
            lsum = sb.tile([128, HALF * 4], DT_F32, tag="lsum")
            nc.scalar.activation(lsum[:], ssum[:, ts0 * 4:(ts0 + HALF) * 4], AF.Ln)
            for t in range(ts0, ts0 + HALF):
                oT = sb.tile([128, 4, NUM_CLASS], DT_F32, tag="oT")
                for pc in range(4):
                    tl = (t - ts0) * 4 + pc
                    nc.vector.tensor_scalar(oT[:, pc, :], zt[:, t, pc, :],
                                            lsum[:, tl:tl + 1], None,
                                            op0=ALU.subtract)
                nc.sync.dma_start(
                    out=lo_d[bass.ds(t * TPTS, TPTS)].rearrange(
                        "(pc p) c -> p pc c", pc=4),
                    in_=oT[:])

    nc.compile()
    return nc


# ---------------------------------------------------------------- entry point

_CACHE = {}


def _run(pos, feat, params, trace=False):
    pos = np.asarray(pos, F32)
    feat = np.asarray(feat, F32)
    wmap = _make_wmap(params)
    if "nc" not in _CACHE:
        _CACHE["nc"] = _build_program(wmap)
    nc = _CACHE["nc"]

    x = np.concatenate([pos, feat], axis=-1)               # (B, N, 9)
    sx = np.sign(x).astype(BF16)
    in_maps = []
    for i in range(M_CORES):
        sl = slice(i * BL, (i + 1) * BL)
        m = {k: v for k, v in wmap.items()}
        m["sxT"] = np.ascontiguousarray(sx[sl].transpose(2, 0, 1).reshape(9, NPTS))
        m["xyzT"] = np.ascontiguousarray(
            pos[sl].transpose(2, 0, 1).reshape(3, NPTS).astype(F32))
        in_maps.append(m)

    res = run_bass_kernel_spmd(nc, in_maps, core_ids=list(range(M_CORES)),
                               trace=trace)
    out = np.concatenate([r["logout"] for r in res.results], axis=0)
    trans = np.concatenate([r["trans_o"] for r in res.results], axis=0)
    tf = np.concatenate([r["tf_o"] for r in res.results], axis=0)
    return (out, trans, tf), res


def kernel(pos, feat, params):
    (out, trans, tf), _ = _run(pos, feat, params, trace=False)
    return out, trans, tf
